# revision 1
# baseline (speedup 1.0000x reference)
"""Trainium2 Bass kernel: GPT-style transformer forward pass.

Strategy: data-parallel over batch across 8 NeuronCores (B=64 -> 8 per core),
weights replicated.  On each core, activations are kept feature-major
(x_T [D=384 (3x128 partitions), 2048 tokens]) so matmuls contract the
partition dim with no activation transposes; LN/QKV/proj/FFN run on
512-token pairs of batch elements.  All matmuls are float32r (full rate at
moving dim >= 256).  LayerNorm stats come from ones-vector matmuls
(partition reduction); rstd = exp(-0.5*ln(var+eps)) on the Scalar engine so
the whole kernel lives in the natural_log_exp activation table set (no
table switches).  Attention computes transposed scores S_T[s,t] per (b,h),
multiplicative causal mask after exp, then token-major PV with an appended
ones-column in V so softmax denominators land as a per-partition column
(cheap [128,6] reciprocal + one broadcast multiply), and PE transposes
carry att back to feature-major for the projection.
"""

import os
import sys

for _p in ("/opt/trn_rl_repo",):
    if _p not in sys.path and os.path.isdir(_p):
        sys.path.insert(0, _p)

import numpy as np

import concourse.bass as bass
import concourse.mybir as mybir
import concourse.tile as tile
from concourse import bacc
from concourse.bass_utils import run_bass_kernel_spmd

F32 = mybir.dt.float32
F32R = mybir.dt.float32r
AF = mybir.ActivationFunctionType
OP = mybir.AluOpType

V, D, H, HD, L, T, B = 65, 384, 6, 64, 6, 256, 64
NCORES = 8
BL = B // NCORES          # batch elements per core
NTOK = BL * T             # tokens per core
DFF = 4 * D               # 1536
EPS = 1e-3
KC = D // 128             # 3 contraction chunks of 128
MT = D // 128             # 3 output feature tiles
FT = DFF // 128           # 12 ffn tiles
HD1 = HD + 2              # V columns incl ones col (64) + pad (65)

USE_F32R = os.environ.get("KERNEL_NO_F32R", "") != "1"
MDT = F32R if USE_F32R else F32

# cst layout: cols 0:512 ones, col 512 = 1/D, cols 513:641 identity 128x128,
# cols 641:769 = 1/D block (stats matmul lhsT)
CST_W = 513 + 128 + 128


def _patch_act_tables():
    """Steer the activation-table picker to natural_log_exp_and_others for
    Exp and Ln, so this kernel's ACT stream never switches table sets.
    Set names/indices are preserved (walrus resolves the same act_info),
    only the picker's view of which sets provide Exp/Ln is narrowed."""
    if getattr(bacc, "_act_tables_patched", False):
        return
    real = bacc.get_activation_tables

    def patched(arch):
        t = real(arch)
        exp = mybir.ActivationFunctionType.Exp
        ln = mybir.ActivationFunctionType.Ln
        out = {}
        for name, fns in t.items():
            if name != "natural_log_exp_and_others":
                fns = fns - {exp, ln}
            out[name] = fns
        return out

    bacc.get_activation_tables = patched
    bacc._act_tables_patched = True


class _MM:
    """matmul emitter with explicit accumulation-chain boundaries."""

    def __init__(self, nc):
        self.nc = nc

    def __call__(self, out, lhsT, rhs, first=True, last=True, tile_position=None):
        self.nc.tensor.matmul(
            out, lhsT, rhs, start=first, stop=last, tile_position=tile_position,
        )


def build_program(n_layers=L, n_b=BL, n_heads=H):
    _patch_act_tables()
    assert n_b % 2 == 0 or n_b == 1
    ntok = n_b * T
    npair = max(1, n_b // 2)
    PW = 512 if n_b > 1 else 256      # tokens per pair-chunk
    nc = bacc.Bacc("TRN2", target_bir_lowering=False, debug=False)

    # ---------------- DRAM I/O ----------------
    hot_d = nc.dram_tensor("hotT", [V, ntok], MDT, kind="ExternalInput").ap()
    temb_d = nc.dram_tensor("temb", [V, D], MDT, kind="ExternalInput").ap()
    pos_d = nc.dram_tensor("posT", [128, KC, 512], F32, kind="ExternalInput").ap()
    mask_d = nc.dram_tensor("mask", [128, 512], F32, kind="ExternalInput").ap()
    wqkv_d = nc.dram_tensor("wqkv", [n_layers, 128, 3, KC, D], MDT, kind="ExternalInput").ap()
    wproj_d = nc.dram_tensor("wproj", [n_layers, 128, KC, D], MDT, kind="ExternalInput").ap()
    w1_d = nc.dram_tensor("w1", [n_layers, 128, KC, DFF], MDT, kind="ExternalInput").ap()
    w2_d = nc.dram_tensor("w2", [n_layers, 128, FT, D], MDT, kind="ExternalInput").ap()
    vbias_d = nc.dram_tensor("vbias", [n_layers, 128, D], F32, kind="ExternalInput").ap()
    biasc_d = nc.dram_tensor("biasc", [n_layers, 128, MT + FT + MT], F32, kind="ExternalInput").ap()
    biasr_d = nc.dram_tensor("biasr", [n_layers, 1, 2 * D], MDT, kind="ExternalInput").ap()
    whead_d = nc.dram_tensor("whead", [128, KC, V], MDT, kind="ExternalInput").ap()
    bhead_d = nc.dram_tensor("bheadc", [V, 1], F32, kind="ExternalInput").ap()
    cst_d = nc.dram_tensor("cst", [128, CST_W], MDT, kind="ExternalInput").ap()
    out_d = nc.dram_tensor("logitsT", [n_b, V, T], F32, kind="ExternalOutput").ap()

    from contextlib import ExitStack

    with tile.TileContext(nc) as tc, \
         nc.allow_low_precision(reason="fp32r matmul operand production"), \
         ExitStack() as ctx:
        ep = ctx.enter_context

        # ---------------- pools ----------------
        cpool = ep(tc.tile_pool(name="consts", bufs=1))
        xpool = ep(tc.tile_pool(name="x", bufs=1))
        wpool_qkv = ep(tc.tile_pool(name="wqkv", bufs=1))
        wpool_proj = ep(tc.tile_pool(name="wproj", bufs=1))
        wpool_1 = ep(tc.tile_pool(name="w1", bufs=1))
        wpool_2 = ep(tc.tile_pool(name="w2", bufs=1))
        wpool_b = ep(tc.tile_pool(name="wbias", bufs=1))
        hpool = ep(tc.tile_pool(name="h", bufs=3))
        xsqpool = ep(tc.tile_pool(name="xsq", bufs=1))
        qpool = ep(tc.tile_pool(name="q", bufs=2))
        kpool = ep(tc.tile_pool(name="k", bufs=2))
        vpool = ep(tc.tile_pool(name="v", bufs=1))
        upool = ep(tc.tile_pool(name="u", bufs=4))
        atmpool = ep(tc.tile_pool(name="atm", bufs=2))
        attpool = ep(tc.tile_pool(name="att", bufs=1))
        h1pool = ep(tc.tile_pool(name="h1", bufs=1))
        lgpool = ep(tc.tile_pool(name="lg", bufs=1))
        stpool = ep(tc.tile_pool(name="st", bufs=6))
        rdpool = ep(tc.tile_pool(name="rd", bufs=4))

        pbig = ep(tc.tile_pool(name="pbig", bufs=4, space="PSUM"))
        ppv = ep(tc.tile_pool(name="ppv", bufs=2, space="PSUM"))
        pstat = ep(tc.tile_pool(name="pstat", bufs=2, space="PSUM"))

        mm = _MM(nc)

        # ---------------- constants ----------------
        cst = cpool.tile([128, CST_W], MDT, name="cst_c")
        nc.sync.dma_start(out=cst[:, :], in_=cst_d[:, :])
        ones = cst[:, 0:512]
        ident = cst[:, 513:641]
        invD = cst[:, 641:769]
        mask = cpool.tile([128, 512], F32, name="mask_c")
        nc.sync.dma_start(out=mask[:, :], in_=mask_d[:, :])
        whead = cpool.tile([128, KC, V], MDT, name="whead_c")
        nc.sync.dma_start(out=whead[:, :, :], in_=whead_d[:, :, :])
        bhead = cpool.tile([V, 1], F32, name="bhead_c")
        nc.sync.dma_start(out=bhead[:, :], in_=bhead_d[:, :])

        x = xpool.tile([128, KC, ntok], MDT, name="x_resid")

        nbp = PW // T
        v_tiles = [vpool.tile([128, 2 * nbp, n_heads, HD1], MDT, name=f"v_pp{i}")
                   for i in range(2)]
        for vt in v_tiles:
            for tb in range(2 * nbp):
                nc.vector.tensor_copy(vt[:, tb, :, HD:HD1],
                                      ones[:, 0:2 * n_heads].rearrange(
                                          "p (h c) -> p h c", h=n_heads))

        # ---------------- embedding (scratch tiles borrow other pools) ----------------
        pos = attpool.tile([128, KC, 512], F32, tag="att")
        nc.sync.dma_start(out=pos[:, :, :], in_=pos_d[:, :, :])
        temb = stpool.tile([V, 384], MDT, tag="st")
        nc.sync.dma_start(out=temb[:, :], in_=temb_d[:, :])
        for ch in range(ntok // PW):
            cs = slice(ch * PW, ch * PW + PW)
            hot = xsqpool.tile([V, PW], MDT, tag="xsq")
            nc.sync.dma_start(out=hot[:, :], in_=hot_d[:, cs])
            for c in range(KC):
                ps = pbig.tile([128, 512], F32, tag="pbig")
                mm(ps[:, 0:PW], temb[0:V, c * 128:(c + 1) * 128], hot[0:V, :])
                nc.vector.tensor_tensor(x[:, c, cs], ps[:, 0:PW], pos[:, c, 0:PW], op=OP.add)

        # ---------------- LN split into stats + finish ----------------
        def ln_stats(p):
            """square + M=128 stat matmuls (mean/meansq arrive pre-broadcast
            across partitions) + the DVE/ACT rstd chain, all off the PE"""
            pc = slice(p * PW, p * PW + PW)
            xsq = xsqpool.tile([128, KC, PW], MDT, tag="xsq")
            nc.scalar.activation(xsq[:, :, :], x[:, :, pc], AF.Square)
            meanb = pstat.tile([128, PW], F32, tag="pstat")
            msqb = pstat.tile([128, PW], F32, tag="pstat")
            for c in range(KC):
                mm(meanb[:, :], invD[:, :], x[:, c, pc],
                   first=(c == 0), last=(c == KC - 1))
            for c in range(KC):
                mm(msqb[:, :], invD[:, :], xsq[:, c, :],
                   first=(c == 0), last=(c == KC - 1))
            m2b = stpool.tile([128, PW], F32, tag="st")
            nc.scalar.activation(m2b[:, :], meanb[:, :], AF.Square)
            varb = stpool.tile([128, PW], F32, tag="st")
            nc.vector.scalar_tensor_tensor(
                varb[:, :], msqb[:, :], EPS, m2b[:, :],
                op0=OP.add, op1=OP.subtract)
            nc.scalar.activation(varb[:, :], varb[:, :], AF.Ln)
            rstdb = stpool.tile([128, PW], MDT, tag="st")
            nc.scalar.activation(rstdb[:, :], varb[:, :], AF.Exp, scale=-0.5)
            mrb = stpool.tile([128, PW], MDT, tag="st")
            nc.vector.tensor_tensor(mrb[:, :], meanb[:, :], rstdb[:, :], op=OP.mult)
            return (p, rstdb, mrb)

        def ln_finish(tok):
            """apply -> h (pure DVE; no PE work)"""
            p, rstdb, mrb = tok
            pc = slice(p * PW, p * PW + PW)
            h = hpool.tile([128, KC, PW], MDT, tag="h")
            for c in range(KC):
                nc.vector.tensor_tensor(h[:, c, :], x[:, c, pc], rstdb[:, :], op=OP.mult)
                nc.vector.tensor_tensor(h[:, c, :], h[:, c, :], mrb[:, :], op=OP.subtract)
            return h

        def run(mids):
            out = []
            for f in mids:
                out.append(f())
            return out

        # ---------------- phase B: qkv + attention + proj for a pair ----------------
        def emit_B(p, h, wqkv, wproj, vbias, biasc, biasr, mid_a=(), mid_b=()):
            pc = slice(p * PW, p * PW + PW)
            nb_in_p = PW // T
            q_t = qpool.tile([128, MT, PW], MDT, tag="q")
            k_t = kpool.tile([128, MT, PW], MDT, tag="k")
            for mat, dst in ((0, q_t), (1, k_t)):
                for mt in range(MT):
                    ps = pbig.tile([128, 512], F32, tag="pbig")
                    for kc in range(KC):
                        mm(ps[:, 0:PW], wqkv[:, mat, kc, mt * 128:(mt + 1) * 128],
                           h[:, kc, :], first=(kc == 0), last=False)
                    mm(ps[:, 0:PW],
                       biasr[0:1, mat * D + mt * 128: mat * D + (mt + 1) * 128],
                       ones[0:1, 0:PW], first=False, last=True)
                    nc.vector.tensor_copy(dst[:, mt, :], ps[:, 0:PW])
            v_t = v_tiles[p % 2]
            for tb in range(2 * nb_in_p):
                vps = pbig.tile([128, 512], F32, tag="pbig")
                for kc in range(KC):
                    mm(vps[:, 0:D], h[:, kc, tb * 128:(tb + 1) * 128],
                       wqkv[:, 2, kc, :], first=(kc == 0), last=(kc == KC - 1))
                nc.vector.tensor_tensor(
                    v_t[:, tb, :, 0:HD],
                    vps[:, 0:D].rearrange("p (h d) -> p h d", h=n_heads),
                    vbias[:, :].rearrange("p (h d) -> p h d", h=n_heads),
                    op=OP.add)
            mids_out = run(mid_a)
            atms = []
            for bi in range(nb_in_p):
                boff = bi * T
                if bi == 1:
                    mids_out += run(mid_b)
                us = [None] * n_heads
                pv0 = ppv.tile([128, n_heads, HD1], F32, tag="ppv")
                pv1 = ppv.tile([128, n_heads, HD1], F32, tag="ppv")

                def emit_S(hh):
                    hp = 64 * (hh % 2)
                    hc = hh // 2
                    sps = pbig.tile([128, 512], F32, tag="pbig")
                    mm(sps[:, 0:256], k_t[hp:hp + HD, hc, boff:boff + 128],
                       q_t[hp:hp + HD, hc, boff:boff + T])
                    mm(sps[:, 256:512], k_t[hp:hp + HD, hc, boff + 128:boff + 256],
                       q_t[hp:hp + HD, hc, boff:boff + T])
                    u_t = upool.tile([128, 512], MDT, tag="u")
                    nc.scalar.activation(u_t[:, :], sps[:, :], AF.Exp)
                    nc.vector.tensor_tensor(u_t[:, :], u_t[:, :], mask[:, :], op=OP.mult)
                    us[hh] = u_t

                def emit_PV(hh):
                    u_t = us[hh]
                    mm(pv0[:, hh, :], u_t[:, 0:128], v_t[:, 2 * bi, hh, :])
                    mm(pv1[:, hh, :], u_t[:, 128:256], v_t[:, 2 * bi, hh, :],
                       first=True, last=False)
                    mm(pv1[:, hh, :], u_t[:, 384:512], v_t[:, 2 * bi + 1, hh, :],
                       first=False, last=True)

                # window the S/PV interleave so only ~3 U tiles are live
                emit_S(0); emit_S(1); emit_S(2)
                emit_PV(0); emit_S(3)
                emit_PV(1); emit_S(4)
                emit_PV(2); emit_S(5)
                emit_PV(3); emit_PV(4); emit_PV(5)
                atm = atmpool.tile([128, 2, n_heads * HD], MDT, tag="atm")
                for tb, pv in ((0, pv0), (1, pv1)):
                    rden = rdpool.tile([128, n_heads], F32, tag="rd")
                    nc.vector.reciprocal(rden[:, :], pv[:, :, HD])
                    nc.vector.tensor_tensor(
                        atm[:, tb, :].rearrange("p (h d) -> p h d", h=n_heads),
                        pv[:, :, 0:HD],
                        rden[:, :, None].broadcast_to([128, n_heads, HD]),
                        op=OP.mult)
                atms.append(atm)
            att_t = attpool.tile([128, KC, PW], MDT, tag="att")
            for c in range(KC):
                tps = pbig.tile([128, 512], MDT, tag="pbig")
                for bi in range(nb_in_p):
                    for tb in range(2):
                        col = (bi * 2 + tb) * 128
                        nc.tensor.transpose(
                            tps[:, col:col + 128],
                            atms[bi][:, tb, c * 128:(c + 1) * 128],
                            ident[:, :])
                nc.vector.tensor_copy(att_t[:, c, :], tps[:, 0:PW])
            for mt in range(MT):
                pp = pbig.tile([128, 512], F32, tag="pbig")
                for kc in range(KC):
                    mm(pp[:, 0:PW], wproj[:, kc, mt * 128:(mt + 1) * 128],
                       att_t[:, kc, :], first=(kc == 0), last=(kc == KC - 1))
                nc.vector.scalar_tensor_tensor(
                    x[:, mt, pc], pp[:, 0:PW], biasc[:, mt:mt + 1], x[:, mt, pc],
                    op0=OP.add, op1=OP.add)
            return mids_out

        # ---------------- phase D: FFN for a pair ----------------
        def emit_D(p, h2, w1, w2, biasc, mid=()):
            pc = slice(p * PW, p * PW + PW)
            h1_t = h1pool.tile([128, FT, PW], MDT, tag="h1")
            for mt in range(FT):
                fps = pbig.tile([128, 512], F32, tag="pbig")
                for kc in range(KC):
                    mm(fps[:, 0:PW], w1[:, kc, mt * 128:(mt + 1) * 128],
                       h2[:, kc, :], first=(kc == 0), last=(kc == KC - 1))
                nc.scalar.activation(h1_t[:, mt, :], fps[:, 0:PW], AF.Relu,
                                     bias=biasc[:, MT + mt:MT + mt + 1])
            mids_out = run(mid)
            for mt in range(MT):
                fp2 = pbig.tile([128, 512], F32, tag="pbig")
                for kc in range(FT):
                    mm(fp2[:, 0:PW], w2[:, kc, mt * 128:(mt + 1) * 128],
                       h1_t[:, kc, :], first=(kc == 0), last=(kc == FT - 1))
                nc.vector.scalar_tensor_tensor(
                    x[:, mt, pc], fp2[:, 0:PW],
                    biasc[:, MT + FT + mt:MT + FT + mt + 1],
                    x[:, mt, pc], op0=OP.add, op1=OP.add)
            return mids_out

        # ---------------- layers: software-pipelined emission ----------------
        carry = {}
        for l in range(n_layers):
            wqkv = wpool_qkv.tile([128, 3, KC, D], MDT, tag="wqkv")
            nc.sync.dma_start(out=wqkv[:, :, :, :], in_=wqkv_d[l])
            wproj = wpool_proj.tile([128, KC, D], MDT, tag="wproj")
            nc.sync.dma_start(out=wproj[:, :, :], in_=wproj_d[l])
            w1 = wpool_1.tile([128, KC, DFF], MDT, tag="w1")
            nc.sync.dma_start(out=w1[:, :, :], in_=w1_d[l])
            w2 = wpool_2.tile([128, FT, D], MDT, tag="w2")
            nc.sync.dma_start(out=w2[:, :, :], in_=w2_d[l])
            vbias = wpool_b.tile([128, D], F32, tag="vbias")
            nc.sync.dma_start(out=vbias[:, :], in_=vbias_d[l])
            biasc = wpool_b.tile([128, MT + FT + MT], F32, tag="biasc")
            nc.sync.dma_start(out=biasc[:, :], in_=biasc_d[l])
            biasr = wpool_b.tile([1, 2 * D], MDT, tag="biasr")
            nc.sync.dma_start(out=biasr[0:1, :], in_=biasr_d[l])

            B = lambda p, h, **kw: emit_B(p, h, wqkv, wproj, vbias, biasc, biasr, **kw)
            Dp = lambda p, h2, **kw: emit_D(p, h2, w1, w2, biasc, **kw)

            if npair == 4:
                if l == 0:
                    h0 = ln_finish(ln_stats(0))
                    s1 = ln_stats(1)
                else:
                    h0, s1 = carry["h0"], carry["s1"]
                (h1,) = B(0, h0, mid_a=[lambda: ln_finish(s1)])
                sc0 = ln_stats(0)
                h2_0, s2 = B(1, h1, mid_a=[lambda: ln_finish(sc0),
                                           lambda: ln_stats(2)])
                (g2,) = Dp(0, h2_0, mid=[lambda: ln_finish(s2)])
                sc1 = ln_stats(1)
                h2_1, s3 = B(2, g2, mid_a=[lambda: ln_finish(sc1),
                                           lambda: ln_stats(3)])
                (g3,) = Dp(1, h2_1, mid=[lambda: ln_finish(s3)])
                sc2 = ln_stats(2)
                (h2_2,) = B(3, g3, mid_a=[lambda: ln_finish(sc2)])
                last = (l == n_layers - 1)
                if not last:
                    sc3, s0n = Dp(2, h2_2, mid=[lambda: ln_stats(3),
                                                lambda: ln_stats(0)])
                    # note: ln_stats(0) here reads x pair0 as updated by D0 above
                    h2_3 = ln_finish(sc3)
                    (h0n,) = Dp(3, h2_3, mid=[lambda: ln_finish(s0n)])
                    carry = {"h0": h0n, "s1": ln_stats(1)}
                else:
                    (sc3,) = Dp(2, h2_2, mid=[lambda: ln_stats(3)])
                    h2_3 = ln_finish(sc3)
                    Dp(3, h2_3)
            else:
                # simple order for small test configs
                hq = {}
                hq[0] = ln_finish(ln_stats(0))
                for p in range(1, npair):
                    hq[p] = ln_finish(ln_stats(p))
                    B(p - 1, hq.pop(p - 1))
                B(npair - 1, hq.pop(npair - 1))
                hq[0] = ln_finish(ln_stats(0))
                for p in range(1, npair):
                    hq[p] = ln_finish(ln_stats(p))
                    Dp(p - 1, hq.pop(p - 1))
                Dp(npair - 1, hq.pop(npair - 1))

        # ---------------- final LN + head ----------------
        for p in range(npair):
            hf = ln_finish(ln_stats(p))
            for bi in range(PW // T):
                b = p * (PW // T) + bi
                hps = ppv.tile([V, 256], F32, tag="ppv")
                for kc in range(KC):
                    mm(hps[:, :], whead[:, kc, :], hf[:, kc, bi * T:(bi + 1) * T],
                       first=(kc == 0), last=(kc == KC - 1))
                lg = lgpool.tile([V, T], F32, tag="lg")
                nc.vector.tensor_scalar(lg[:, :], hps[:, :], bhead[0:V, 0:1], None,
                                        op0=OP.add)
                nc.sync.dma_start(out=out_d[b], in_=lg[:, :])

    nc.compile()
    return nc


# ---------------------------------------------------------------------------
# host side
# ---------------------------------------------------------------------------

def prep_inputs(inputs, n_layers=L, n_b=BL, core=0):
    """Build the per-core input map (numpy) for `core`."""
    f32 = np.float32
    idx = np.asarray(inputs["idx"])
    tok_emb = np.asarray(inputs["tok_emb"], f32)
    pos_emb = np.asarray(inputs["pos_emb"], f32)
    Wq = np.asarray(inputs["Wq"], f32)
    Wk = np.asarray(inputs["Wk"], f32)
    Wv = np.asarray(inputs["Wv"], f32)
    Wproj = np.asarray(inputs["Wproj"], f32)
    bproj = np.asarray(inputs["bproj"], f32)
    W1 = np.asarray(inputs["W1"], f32)
    b1 = np.asarray(inputs["b1"], f32)
    W2 = np.asarray(inputs["W2"], f32)
    b2 = np.asarray(inputs["b2"], f32)
    ln1_g = np.asarray(inputs["ln1_g"], f32)
    ln1_b = np.asarray(inputs["ln1_b"], f32)
    ln2_g = np.asarray(inputs["ln2_g"], f32)
    ln2_b = np.asarray(inputs["ln2_b"], f32)
    lnf_g = np.asarray(inputs["lnf_g"], f32)
    lnf_b = np.asarray(inputs["lnf_b"], f32)
    Whead = np.asarray(inputs["Whead"], f32)
    bhead = np.asarray(inputs["bhead"], f32)

    ntok = n_b * T
    scale = f32(D) ** -0.5

    idx_c = idx[core * n_b:(core + 1) * n_b].reshape(-1)         # [ntok]
    hot = (idx_c[None, :] == np.arange(V)[:, None]).astype(f32)  # [V, ntok]

    posT = pos_emb.T.astype(f32)                                 # [D, T]
    posT2 = np.concatenate([posT, posT], axis=1)                 # [D, 512]
    pos_in = posT2.reshape(KC, 128, 512).transpose(1, 0, 2).copy()

    lane = np.arange(128)
    t = np.arange(T)
    m0 = (lane[:, None] <= t[None, :]).astype(f32)
    m1 = ((lane[:, None] + 128) <= t[None, :]).astype(f32)
    mask = np.concatenate([m0, m1], axis=1)                      # [128, 512]

    def pack_w(w):  # [D_in, N] -> [128, KC_in, N]
        kin = w.shape[0] // 128
        return w.reshape(kin, 128, -1).transpose(1, 0, 2).copy()

    wqkv = np.zeros((n_layers, 128, 3, KC, D), f32)
    wproj = np.zeros((n_layers, 128, KC, D), f32)
    w1 = np.zeros((n_layers, 128, KC, DFF), f32)
    w2 = np.zeros((n_layers, 128, FT, D), f32)
    vbias = np.zeros((n_layers, 128, D), f32)
    biasc = np.zeros((n_layers, 128, MT + FT + MT), f32)
    biasr = np.zeros((n_layers, 1, 2 * D), f32)

    for l in range(n_layers):
        # Wq[l] is [H, D, HD]; feature f = h*HD+hd -> transpose to [D, H, HD]
        wq2 = Wq[l].transpose(1, 0, 2).reshape(D, D) * scale
        wk2 = Wk[l].transpose(1, 0, 2).reshape(D, D)
        wv2 = Wv[l].transpose(1, 0, 2).reshape(D, D)
        wqkv[l, :, 0] = pack_w(wq2 * ln1_g[l][:, None])
        wqkv[l, :, 1] = pack_w(wk2 * ln1_g[l][:, None])
        wqkv[l, :, 2] = pack_w(wv2 * ln1_g[l][:, None])
        biasr[l, 0, 0:D] = ln1_b[l] @ wq2
        biasr[l, 0, D:2 * D] = ln1_b[l] @ wk2
        vbias[l] = np.broadcast_to(ln1_b[l] @ wv2, (128, D))
        wproj[l] = pack_w(Wproj[l])
        w1[l] = pack_w(W1[l] * ln2_g[l][:, None])
        w2[l] = pack_w(W2[l])
        biasc[l, :, 0:MT] = bproj[l].reshape(MT, 128).T
        biasc[l, :, MT:MT + FT] = (b1[l] + ln2_b[l] @ W1[l]).reshape(FT, 128).T
        biasc[l, :, MT + FT:] = b2[l].reshape(MT, 128).T

    whead_eff = Whead * lnf_g[:, None]
    bhead_eff = (bhead + lnf_b @ Whead).astype(f32)

    cst = np.ones((128, CST_W), f32)
    cst[:, 512] = 1.0 / D
    cst[:, 513:641] = np.eye(128, dtype=f32)
    cst[:, 641:769] = 1.0 / D

    return {
        "cst": cst,
        "hotT": hot,
        "temb": tok_emb.astype(f32),
        "posT": pos_in,
        "mask": mask,
        "wqkv": wqkv,
        "wproj": wproj,
        "w1": w1,
        "w2": w2,
        "vbias": vbias,
        "biasc": biasc,
        "biasr": biasr,
        "whead": pack_w(whead_eff),
        "bheadc": bhead_eff[:, None].copy(),
    }


_CACHE = {}


def get_program():
    if "nc" not in _CACHE:
        _CACHE["nc"] = build_program()
    return _CACHE["nc"]


def run_on_hw(inputs, trace=False):
    nc = get_program()
    in_maps = [prep_inputs(inputs, core=c) for c in range(NCORES)]
    res = run_bass_kernel_spmd(nc, in_maps, list(range(NCORES)), trace=trace)
    outs = []
    for c in range(NCORES):
        lt = res.results[c]["logitsT"]          # [BL, V, T]
        outs.append(lt.transpose(0, 2, 1))      # [BL, T, V]
    full = np.concatenate(outs, axis=0)         # [B, T, V]
    return full, res


def kernel(**inputs):
    out, _ = run_on_hw(inputs, trace=False)
    return out



# revision 2
# speedup vs baseline: 1.1583x; 1.1583x over previous
"""Trainium2 Bass kernel: GPT-style transformer forward pass.

Strategy: data-parallel over batch across 8 NeuronCores (B=64 -> 8 per core),
weights replicated.  On each core, activations are kept feature-major
(x_T [D=384 (3x128 partitions), 2048 tokens]) so matmuls contract the
partition dim with no activation transposes; LN/QKV/proj/FFN run on
512-token pairs of batch elements.  All matmul operands are bfloat16 (PSUM
accumulation stays fp32): on TRN2 hardware bf16 streams the PE at 2.4 GHz
while fp32r runs at the 1.2 GHz "others" clock, so bf16 doubles matmul
throughput on top of halving LDWEIGHTS and weight DMA.  The fp32 residual
stream x is kept in SBUF; a bf16 shadow is cast on the (otherwise idle)
GPSIMD engine for the LN mean matmuls.  LayerNorm stats come from
ones-vector matmuls (partition reduction); rstd = exp(-0.5*ln(var+eps)) on
the Scalar engine so the whole kernel lives in the natural_log_exp
activation table set.  Attention computes transposed scores S_T[s,t] per
(b,h) for the three live causal blocks only, exp on Scalar, one grouped
multiplicative mask per 3 heads on Vector, then token-major PV with an
appended ones-column in V so softmax denominators land as a per-partition
column, and PE transposes carry att back to feature-major for the
projection.  Q/K biases ride the PSUM->SBUF copy as tensor_scalar adds
instead of rank-1 matmuls.
"""

import os
import sys

for _p in ("/opt/trn_rl_repo",):
    if _p not in sys.path and os.path.isdir(_p):
        sys.path.insert(0, _p)

import numpy as np
import ml_dtypes

import concourse.bass as bass
import concourse.mybir as mybir
import concourse.tile as tile
from concourse import bacc
from concourse.bass_utils import run_bass_kernel_spmd

F32 = mybir.dt.float32
BF16 = mybir.dt.bfloat16
NPBF = ml_dtypes.bfloat16
AF = mybir.ActivationFunctionType
OP = mybir.AluOpType

V, D, H, HD, L, T, B = 65, 384, 6, 64, 6, 256, 64
NCORES = 8
BL = B // NCORES          # batch elements per core
NTOK = BL * T             # tokens per core
DFF = 4 * D               # 1536
EPS = 1e-3
KC = D // 128             # 3 contraction chunks of 128
MT = D // 128             # 3 output feature tiles
FT = DFF // 128           # 12 ffn tiles
HD1 = HD + 2              # V columns incl ones col (64) + pad (65)
NBC = MT + FT + MT + MT + MT  # biasc cols: proj, ffn1, ffn2, q, k

MDT = BF16

# cst layout: cols 0:512 ones, col 512 = 1/D, cols 513:641 identity 128x128,
# cols 641:769 = 1/D block (stats matmul lhsT)
CST_W = 513 + 128 + 128


def _patch_act_tables():
    """Steer the activation-table picker to natural_log_exp_and_others for
    Exp and Ln, so this kernel's ACT stream never switches table sets."""
    if getattr(bacc, "_act_tables_patched", False):
        return
    real = bacc.get_activation_tables

    def patched(arch):
        t = real(arch)
        exp = mybir.ActivationFunctionType.Exp
        ln = mybir.ActivationFunctionType.Ln
        out = {}
        for name, fns in t.items():
            if name != "natural_log_exp_and_others":
                fns = fns - {exp, ln}
            out[name] = fns
        return out

    bacc.get_activation_tables = patched
    bacc._act_tables_patched = True


class _MM:
    """matmul emitter with explicit accumulation-chain boundaries."""

    def __init__(self, nc):
        self.nc = nc

    def __call__(self, out, lhsT, rhs, first=True, last=True, tile_position=None):
        self.nc.tensor.matmul(
            out, lhsT, rhs, start=first, stop=last, tile_position=tile_position,
        )


def build_program(n_layers=L, n_b=BL, n_heads=H):
    _patch_act_tables()
    assert n_b % 2 == 0 or n_b == 1
    ntok = n_b * T
    npair = max(1, n_b // 2)
    PW = 512 if n_b > 1 else 256      # tokens per pair-chunk
    nc = bacc.Bacc("TRN2", target_bir_lowering=False, debug=False)

    # ---------------- DRAM I/O ----------------
    hot_d = nc.dram_tensor("hotT", [V, ntok], MDT, kind="ExternalInput").ap()
    temb_d = nc.dram_tensor("temb", [V, D], MDT, kind="ExternalInput").ap()
    pos_d = nc.dram_tensor("posT", [128, KC, 512], MDT, kind="ExternalInput").ap()
    mask_d = nc.dram_tensor("mask", [128, 384], MDT, kind="ExternalInput").ap()
    wqkv_d = nc.dram_tensor("wqkv", [n_layers, 128, 3, KC, D], MDT, kind="ExternalInput").ap()
    wproj_d = nc.dram_tensor("wproj", [n_layers, 128, KC, D], MDT, kind="ExternalInput").ap()
    w1_d = nc.dram_tensor("w1", [n_layers, 128, KC, DFF], MDT, kind="ExternalInput").ap()
    w2_d = nc.dram_tensor("w2", [n_layers, 128, FT, D], MDT, kind="ExternalInput").ap()
    vbias_d = nc.dram_tensor("vbias", [n_layers, 128, D], F32, kind="ExternalInput").ap()
    biasc_d = nc.dram_tensor("biasc", [n_layers, 128, NBC], F32, kind="ExternalInput").ap()
    whead_d = nc.dram_tensor("whead", [128, KC, V], MDT, kind="ExternalInput").ap()
    bhead_d = nc.dram_tensor("bheadc", [V, 1], F32, kind="ExternalInput").ap()
    cst_d = nc.dram_tensor("cst", [128, CST_W], MDT, kind="ExternalInput").ap()
    out_d = nc.dram_tensor("logitsT", [n_b, V, T], F32, kind="ExternalOutput").ap()

    from contextlib import ExitStack

    with tile.TileContext(nc) as tc, \
         nc.allow_low_precision(reason="bf16 matmul operand production"), \
         ExitStack() as ctx:
        ep = ctx.enter_context

        # ---------------- pools ----------------
        cpool = ep(tc.tile_pool(name="consts", bufs=1))
        xpool = ep(tc.tile_pool(name="x", bufs=1))
        wpool_qkv = ep(tc.tile_pool(name="wqkv", bufs=1))
        wpool_proj = ep(tc.tile_pool(name="wproj", bufs=1))
        wpool_1 = ep(tc.tile_pool(name="w1", bufs=1))
        wpool_2 = ep(tc.tile_pool(name="w2", bufs=1))
        wpool_b = ep(tc.tile_pool(name="wbias", bufs=1))
        hpool = ep(tc.tile_pool(name="h", bufs=3))
        xsqpool = ep(tc.tile_pool(name="xsq", bufs=2))
        xbpool = ep(tc.tile_pool(name="xb", bufs=2))
        qpool = ep(tc.tile_pool(name="q", bufs=2))
        kpool = ep(tc.tile_pool(name="k", bufs=2))
        vpool = ep(tc.tile_pool(name="v", bufs=1))
        upool = ep(tc.tile_pool(name="u", bufs=2))
        atmpool = ep(tc.tile_pool(name="atm", bufs=2))
        attpool = ep(tc.tile_pool(name="att", bufs=1))
        h1pool = ep(tc.tile_pool(name="h1", bufs=1))
        lgpool = ep(tc.tile_pool(name="lg", bufs=1))
        stpool = ep(tc.tile_pool(name="st", bufs=6))
        rdpool = ep(tc.tile_pool(name="rd", bufs=4))

        pbig = ep(tc.tile_pool(name="pbig", bufs=4, space="PSUM"))
        ppv = ep(tc.tile_pool(name="ppv", bufs=2, space="PSUM"))
        pstat = ep(tc.tile_pool(name="pstat", bufs=2, space="PSUM"))

        mm = _MM(nc)

        # ---------------- constants ----------------
        cst = cpool.tile([128, CST_W], MDT, name="cst_c")
        nc.sync.dma_start(out=cst[:, :], in_=cst_d[:, :])
        ones = cst[:, 0:512]
        ident = cst[:, 513:641]
        invD = cst[:, 641:769]
        mask = cpool.tile([128, 384], MDT, name="mask_c")
        nc.sync.dma_start(out=mask[:, :], in_=mask_d[:, :])
        whead = cpool.tile([128, KC, V], MDT, name="whead_c")
        nc.sync.dma_start(out=whead[:, :, :], in_=whead_d[:, :, :])
        bhead = cpool.tile([V, 1], F32, name="bhead_c")
        nc.sync.dma_start(out=bhead[:, :], in_=bhead_d[:, :])

        x = xpool.tile([128, KC, ntok], F32, name="x_resid")

        nbp = PW // T
        v_tiles = [vpool.tile([128, 2 * nbp, n_heads, HD1], MDT, name=f"v_pp{i}")
                   for i in range(2)]
        for vt in v_tiles:
            for tb in range(2 * nbp):
                nc.vector.tensor_copy(vt[:, tb, :, HD:HD1],
                                      ones[:, 0:2 * n_heads].rearrange(
                                          "p (h c) -> p h c", h=n_heads))

        # ---------------- embedding (scratch tiles borrow other pools) ----------------
        pos = attpool.tile([128, KC, 512], MDT, tag="att")
        nc.sync.dma_start(out=pos[:, :, :], in_=pos_d[:, :, :])
        temb = stpool.tile([V, 384], MDT, tag="st")
        nc.sync.dma_start(out=temb[:, :], in_=temb_d[:, :])
        for ch in range(ntok // PW):
            cs = slice(ch * PW, ch * PW + PW)
            hot = xsqpool.tile([V, PW], MDT, tag="xsq")
            nc.sync.dma_start(out=hot[:, :], in_=hot_d[:, cs])
            for c in range(KC):
                ps = pbig.tile([128, 512], F32, tag="pbig")
                mm(ps[:, 0:PW], temb[0:V, c * 128:(c + 1) * 128], hot[0:V, :])
                nc.vector.tensor_tensor(x[:, c, cs], ps[:, 0:PW], pos[:, c, 0:PW], op=OP.add)

        # ---------------- LN split into stats + finish ----------------
        def ln_stats(p):
            """bf16 shadow (GPSIMD) + square (ACT) + M=128 stat matmuls
            (mean/meansq arrive pre-broadcast across partitions) + the
            DVE/ACT rstd chain, all off the PE's critical data path"""
            pc = slice(p * PW, p * PW + PW)
            xb = xbpool.tile([128, KC, PW], MDT, tag="xb")
            nc.gpsimd.tensor_copy(xb[:, :, :], x[:, :, pc])
            xsq = xsqpool.tile([128, KC, PW], MDT, tag="xsq")
            nc.scalar.activation(xsq[:, :, :], x[:, :, pc], AF.Square)
            meanb = pstat.tile([128, PW], F32, tag="pstat")
            msqb = pstat.tile([128, PW], F32, tag="pstat")
            for c in range(KC):
                mm(meanb[:, :], invD[:, :], xb[:, c, :],
                   first=(c == 0), last=(c == KC - 1))
            for c in range(KC):
                mm(msqb[:, :], invD[:, :], xsq[:, c, :],
                   first=(c == 0), last=(c == KC - 1))
            m2b = stpool.tile([128, PW], F32, tag="st")
            nc.scalar.activation(m2b[:, :], meanb[:, :], AF.Square)
            varb = stpool.tile([128, PW], F32, tag="st")
            nc.vector.scalar_tensor_tensor(
                varb[:, :], msqb[:, :], EPS, m2b[:, :],
                op0=OP.add, op1=OP.subtract)
            nc.scalar.activation(varb[:, :], varb[:, :], AF.Ln)
            rstdb = stpool.tile([128, PW], F32, tag="st")
            nc.scalar.activation(rstdb[:, :], varb[:, :], AF.Exp, scale=-0.5)
            mrb = stpool.tile([128, PW], MDT, tag="st")
            nc.vector.tensor_tensor(mrb[:, :], meanb[:, :], rstdb[:, :], op=OP.mult)
            return (p, rstdb, mrb)

        def ln_finish(tok):
            """apply -> h (pure DVE; no PE work)"""
            p, rstdb, mrb = tok
            pc = slice(p * PW, p * PW + PW)
            h = hpool.tile([128, KC, PW], MDT, tag="h")
            nc.vector.tensor_tensor(
                h[:, :, :], x[:, :, pc],
                rstdb[:, None, :].broadcast_to([128, KC, PW]), op=OP.mult)
            nc.vector.tensor_tensor(
                h[:, :, :], h[:, :, :],
                mrb[:, None, :].broadcast_to([128, KC, PW]), op=OP.subtract)
            return h

        def run(mids):
            out = []
            for f in mids:
                out.append(f())
            return out

        # ---------------- phase B: qkv + attention + proj for a pair ----------------
        def emit_B(p, h, wqkv, wproj, vbias, biasc, mid_a=(), mid_b=()):
            pc = slice(p * PW, p * PW + PW)
            nb_in_p = PW // T
            q_t = qpool.tile([128, MT, PW], MDT, tag="q")
            k_t = kpool.tile([128, MT, PW], MDT, tag="k")
            for mat, dst, bcol in ((0, q_t, MT + FT + MT), (1, k_t, MT + FT + 2 * MT)):
                for mt in range(MT):
                    ps = pbig.tile([128, 512], F32, tag="pbig")
                    for kc in range(KC):
                        mm(ps[:, 0:PW], wqkv[:, mat, kc, mt * 128:(mt + 1) * 128],
                           h[:, kc, :], first=(kc == 0), last=(kc == KC - 1))
                    nc.vector.tensor_scalar(
                        dst[:, mt, :], ps[:, 0:PW],
                        biasc[:, bcol + mt:bcol + mt + 1], None, op0=OP.add)
            v_t = v_tiles[p % 2]
            for tb in range(2 * nb_in_p):
                vps = pbig.tile([128, 512], F32, tag="pbig")
                for kc in range(KC):
                    mm(vps[:, 0:D], h[:, kc, tb * 128:(tb + 1) * 128],
                       wqkv[:, 2, kc, :], first=(kc == 0), last=(kc == KC - 1))
                nc.vector.tensor_tensor(
                    v_t[:, tb, :, 0:HD],
                    vps[:, 0:D].rearrange("p (h d) -> p h d", h=n_heads),
                    vbias[:, :].rearrange("p (h d) -> p h d", h=n_heads),
                    op=OP.add)
            mids_out = run(mid_a)
            atms = []
            for bi in range(nb_in_p):
                boff = bi * T
                if bi == 1:
                    mids_out += run(mid_b)
                us = upool.tile([128, n_heads, 384], MDT, tag="u")
                pv0 = ppv.tile([128, n_heads, HD1], F32, tag="ppv")
                pv1 = ppv.tile([128, n_heads, HD1], F32, tag="ppv")

                def emit_S(hh):
                    # us[:, hh, 0:256]  = S_T[s in chunk0, t 0:256]
                    # us[:, hh, 256:384] = S_T[s in chunk1, t 128:256]
                    hp = 64 * (hh % 2)
                    hc = hh // 2
                    sps = pbig.tile([128, 384], F32, tag="pbig")
                    mm(sps[:, 0:256], k_t[hp:hp + HD, hc, boff:boff + 128],
                       q_t[hp:hp + HD, hc, boff:boff + T])
                    mm(sps[:, 256:384], k_t[hp:hp + HD, hc, boff + 128:boff + 256],
                       q_t[hp:hp + HD, hc, boff + 128:boff + 256])
                    nc.scalar.activation(us[:, hh, :], sps[:, :], AF.Exp)

                def emit_mask(h0):
                    # zero the masked upper-triangles of the two diagonal
                    # blocks (cols 0:128 and 256:384); cols 128:256 are the
                    # fully-live (s chunk0, t 128:256) block (mask==1 there)
                    nc.vector.tensor_tensor(
                        us[:, h0:h0 + 3, :], us[:, h0:h0 + 3, :],
                        mask[:, None, :].broadcast_to([128, 3, 384]), op=OP.mult)

                def emit_PV(hh):
                    mm(pv0[:, hh, :], us[:, hh, 0:128], v_t[:, 2 * bi, hh, :])
                    mm(pv1[:, hh, :], us[:, hh, 128:256], v_t[:, 2 * bi, hh, :],
                       first=True, last=False)
                    mm(pv1[:, hh, :], us[:, hh, 256:384], v_t[:, 2 * bi + 1, hh, :],
                       first=False, last=True)

                emit_S(0); emit_S(1); emit_S(2)
                emit_mask(0)
                emit_PV(0); emit_S(3)
                emit_PV(1); emit_S(4)
                emit_PV(2); emit_S(5)
                emit_mask(3)
                emit_PV(3); emit_PV(4); emit_PV(5)
                atm = atmpool.tile([128, 2, n_heads * HD], MDT, tag="atm")
                for tb, pv in ((0, pv0), (1, pv1)):
                    rden = rdpool.tile([128, n_heads], F32, tag="rd")
                    nc.vector.reciprocal(rden[:, :], pv[:, :, HD])
                    nc.vector.tensor_tensor(
                        atm[:, tb, :].rearrange("p (h d) -> p h d", h=n_heads),
                        pv[:, :, 0:HD],
                        rden[:, :, None].broadcast_to([128, n_heads, HD]),
                        op=OP.mult)
                atms.append(atm)
            att_t = attpool.tile([128, KC, PW], MDT, tag="att")
            for c in range(KC):
                tps = pbig.tile([128, 512], MDT, tag="pbig")
                for bi in range(nb_in_p):
                    for tb in range(2):
                        col = (bi * 2 + tb) * 128
                        nc.tensor.transpose(
                            tps[:, col:col + 128],
                            atms[bi][:, tb, c * 128:(c + 1) * 128],
                            ident[:, :])
                nc.vector.tensor_copy(att_t[:, c, :], tps[:, 0:PW])
            for mt in range(MT):
                pp = pbig.tile([128, 512], F32, tag="pbig")
                for kc in range(KC):
                    mm(pp[:, 0:PW], wproj[:, kc, mt * 128:(mt + 1) * 128],
                       att_t[:, kc, :], first=(kc == 0), last=(kc == KC - 1))
                nc.vector.scalar_tensor_tensor(
                    x[:, mt, pc], pp[:, 0:PW], biasc[:, mt:mt + 1], x[:, mt, pc],
                    op0=OP.add, op1=OP.add)
            return mids_out

        # ---------------- phase D: FFN for a pair ----------------
        def emit_D(p, h2, w1, w2, biasc, mid=()):
            pc = slice(p * PW, p * PW + PW)
            h1_t = h1pool.tile([128, FT, PW], MDT, tag="h1")
            for mt in range(FT):
                fps = pbig.tile([128, 512], F32, tag="pbig")
                for kc in range(KC):
                    mm(fps[:, 0:PW], w1[:, kc, mt * 128:(mt + 1) * 128],
                       h2[:, kc, :], first=(kc == 0), last=(kc == KC - 1))
                nc.scalar.activation(h1_t[:, mt, :], fps[:, 0:PW], AF.Relu,
                                     bias=biasc[:, MT + mt:MT + mt + 1])
            mids_out = run(mid)
            for mt in range(MT):
                fp2 = pbig.tile([128, 512], F32, tag="pbig")
                for kc in range(FT):
                    mm(fp2[:, 0:PW], w2[:, kc, mt * 128:(mt + 1) * 128],
                       h1_t[:, kc, :], first=(kc == 0), last=(kc == FT - 1))
                nc.vector.scalar_tensor_tensor(
                    x[:, mt, pc], fp2[:, 0:PW],
                    biasc[:, MT + FT + mt:MT + FT + mt + 1],
                    x[:, mt, pc], op0=OP.add, op1=OP.add)
            return mids_out

        # ---------------- layers: software-pipelined emission ----------------
        carry = {}
        for l in range(n_layers):
            wqkv = wpool_qkv.tile([128, 3, KC, D], MDT, tag="wqkv")
            nc.sync.dma_start(out=wqkv[:, :, :, :], in_=wqkv_d[l])
            wproj = wpool_proj.tile([128, KC, D], MDT, tag="wproj")
            nc.sync.dma_start(out=wproj[:, :, :], in_=wproj_d[l])
            w1 = wpool_1.tile([128, KC, DFF], MDT, tag="w1")
            nc.sync.dma_start(out=w1[:, :, :], in_=w1_d[l])
            w2 = wpool_2.tile([128, FT, D], MDT, tag="w2")
            nc.sync.dma_start(out=w2[:, :, :], in_=w2_d[l])
            vbias = wpool_b.tile([128, D], F32, tag="vbias")
            nc.sync.dma_start(out=vbias[:, :], in_=vbias_d[l])
            biasc = wpool_b.tile([128, NBC], F32, tag="biasc")
            nc.sync.dma_start(out=biasc[:, :], in_=biasc_d[l])

            Bp = lambda p, h, **kw: emit_B(p, h, wqkv, wproj, vbias, biasc, **kw)
            Dp = lambda p, h2, **kw: emit_D(p, h2, w1, w2, biasc, **kw)

            if npair == 4:
                if l == 0:
                    h0 = ln_finish(ln_stats(0))
                    s1 = ln_stats(1)
                else:
                    h0, s1 = carry["h0"], carry["s1"]
                (h1,) = Bp(0, h0, mid_a=[lambda: ln_finish(s1)])
                sc0 = ln_stats(0)
                h2_0, s2 = Bp(1, h1, mid_a=[lambda: ln_finish(sc0),
                                            lambda: ln_stats(2)])
                (g2,) = Dp(0, h2_0, mid=[lambda: ln_finish(s2)])
                sc1 = ln_stats(1)
                h2_1, s3 = Bp(2, g2, mid_a=[lambda: ln_finish(sc1),
                                            lambda: ln_stats(3)])
                (g3,) = Dp(1, h2_1, mid=[lambda: ln_finish(s3)])
                sc2 = ln_stats(2)
                (h2_2,) = Bp(3, g3, mid_a=[lambda: ln_finish(sc2)])
                last = (l == n_layers - 1)
                if not last:
                    sc3, s0n = Dp(2, h2_2, mid=[lambda: ln_stats(3),
                                                lambda: ln_stats(0)])
                    # note: ln_stats(0) here reads x pair0 as updated by D0 above
                    h2_3 = ln_finish(sc3)
                    (h0n,) = Dp(3, h2_3, mid=[lambda: ln_finish(s0n)])
                    carry = {"h0": h0n, "s1": ln_stats(1)}
                else:
                    (sc3,) = Dp(2, h2_2, mid=[lambda: ln_stats(3)])
                    h2_3 = ln_finish(sc3)
                    Dp(3, h2_3)
            else:
                # simple order for small test configs
                hq = {}
                hq[0] = ln_finish(ln_stats(0))
                for p in range(1, npair):
                    hq[p] = ln_finish(ln_stats(p))
                    Bp(p - 1, hq.pop(p - 1))
                Bp(npair - 1, hq.pop(npair - 1))
                hq[0] = ln_finish(ln_stats(0))
                for p in range(1, npair):
                    hq[p] = ln_finish(ln_stats(p))
                    Dp(p - 1, hq.pop(p - 1))
                Dp(npair - 1, hq.pop(npair - 1))

        # ---------------- final LN + head ----------------
        for p in range(npair):
            hf = ln_finish(ln_stats(p))
            for bi in range(PW // T):
                b = p * (PW // T) + bi
                hps = ppv.tile([V, 256], F32, tag="ppv")
                for kc in range(KC):
                    mm(hps[:, :], whead[:, kc, :], hf[:, kc, bi * T:(bi + 1) * T],
                       first=(kc == 0), last=(kc == KC - 1))
                lg = lgpool.tile([V, T], F32, tag="lg")
                nc.vector.tensor_scalar(lg[:, :], hps[:, :], bhead[0:V, 0:1], None,
                                        op0=OP.add)
                nc.sync.dma_start(out=out_d[b], in_=lg[:, :])

    nc.compile()
    return nc


# ---------------------------------------------------------------------------
# host side
# ---------------------------------------------------------------------------

def prep_inputs(inputs, n_layers=L, n_b=BL, core=0):
    """Build the per-core input map (numpy) for `core`."""
    f32 = np.float32
    idx = np.asarray(inputs["idx"])
    tok_emb = np.asarray(inputs["tok_emb"], f32)
    pos_emb = np.asarray(inputs["pos_emb"], f32)
    Wq = np.asarray(inputs["Wq"], f32)
    Wk = np.asarray(inputs["Wk"], f32)
    Wv = np.asarray(inputs["Wv"], f32)
    Wproj = np.asarray(inputs["Wproj"], f32)
    bproj = np.asarray(inputs["bproj"], f32)
    W1 = np.asarray(inputs["W1"], f32)
    b1 = np.asarray(inputs["b1"], f32)
    W2 = np.asarray(inputs["W2"], f32)
    b2 = np.asarray(inputs["b2"], f32)
    ln1_g = np.asarray(inputs["ln1_g"], f32)
    ln1_b = np.asarray(inputs["ln1_b"], f32)
    ln2_g = np.asarray(inputs["ln2_g"], f32)
    ln2_b = np.asarray(inputs["ln2_b"], f32)
    lnf_g = np.asarray(inputs["lnf_g"], f32)
    lnf_b = np.asarray(inputs["lnf_b"], f32)
    Whead = np.asarray(inputs["Whead"], f32)
    bhead = np.asarray(inputs["bhead"], f32)

    ntok = n_b * T
    scale = f32(D) ** -0.5

    idx_c = idx[core * n_b:(core + 1) * n_b].reshape(-1)         # [ntok]
    hot = (idx_c[None, :] == np.arange(V)[:, None]).astype(f32)  # [V, ntok]

    posT = pos_emb.T.astype(f32)                                 # [D, T]
    posT2 = np.concatenate([posT, posT], axis=1)                 # [D, 512]
    pos_in = posT2.reshape(KC, 128, 512).transpose(1, 0, 2).copy()

    lane = np.arange(128)
    t = np.arange(T)
    tri0 = (lane[:, None] <= t[None, :128]).astype(f32)          # diag block
    mask = np.concatenate([tri0, np.ones((128, 128), f32), tri0], axis=1)

    def pack_w(w):  # [D_in, N] -> [128, KC_in, N]
        kin = w.shape[0] // 128
        return w.reshape(kin, 128, -1).transpose(1, 0, 2).copy()

    wqkv = np.zeros((n_layers, 128, 3, KC, D), f32)
    wproj = np.zeros((n_layers, 128, KC, D), f32)
    w1 = np.zeros((n_layers, 128, KC, DFF), f32)
    w2 = np.zeros((n_layers, 128, FT, D), f32)
    vbias = np.zeros((n_layers, 128, D), f32)
    biasc = np.zeros((n_layers, 128, NBC), f32)

    for l in range(n_layers):
        # Wq[l] is [H, D, HD]; feature f = h*HD+hd -> transpose to [D, H, HD]
        wq2 = Wq[l].transpose(1, 0, 2).reshape(D, D) * scale
        wk2 = Wk[l].transpose(1, 0, 2).reshape(D, D)
        wv2 = Wv[l].transpose(1, 0, 2).reshape(D, D)
        wqkv[l, :, 0] = pack_w(wq2 * ln1_g[l][:, None])
        wqkv[l, :, 1] = pack_w(wk2 * ln1_g[l][:, None])
        wqkv[l, :, 2] = pack_w(wv2 * ln1_g[l][:, None])
        vbias[l] = np.broadcast_to(ln1_b[l] @ wv2, (128, D))
        wproj[l] = pack_w(Wproj[l])
        w1[l] = pack_w(W1[l] * ln2_g[l][:, None])
        w2[l] = pack_w(W2[l])
        biasc[l, :, 0:MT] = bproj[l].reshape(MT, 128).T
        biasc[l, :, MT:MT + FT] = (b1[l] + ln2_b[l] @ W1[l]).reshape(FT, 128).T
        biasc[l, :, MT + FT:MT + FT + MT] = b2[l].reshape(MT, 128).T
        biasc[l, :, MT + FT + MT:MT + FT + 2 * MT] = \
            (ln1_b[l] @ wq2).reshape(MT, 128).T
        biasc[l, :, MT + FT + 2 * MT:] = (ln1_b[l] @ wk2).reshape(MT, 128).T

    whead_eff = Whead * lnf_g[:, None]
    bhead_eff = (bhead + lnf_b @ Whead).astype(f32)

    cst = np.ones((128, CST_W), f32)
    cst[:, 512] = 1.0 / D
    cst[:, 513:641] = np.eye(128, dtype=f32)
    cst[:, 641:769] = 1.0 / D

    bf = lambda a: np.ascontiguousarray(a).astype(NPBF)

    return {
        "cst": bf(cst),
        "hotT": bf(hot),
        "temb": bf(tok_emb),
        "posT": bf(pos_in),
        "mask": bf(mask),
        "wqkv": bf(wqkv),
        "wproj": bf(wproj),
        "w1": bf(w1),
        "w2": bf(w2),
        "vbias": vbias,
        "biasc": biasc,
        "whead": bf(pack_w(whead_eff)),
        "bheadc": bhead_eff[:, None].copy(),
    }


_CACHE = {}


def get_program():
    if "nc" not in _CACHE:
        _CACHE["nc"] = build_program()
    return _CACHE["nc"]


def run_on_hw(inputs, trace=False):
    nc = get_program()
    in_maps = [prep_inputs(inputs, core=c) for c in range(NCORES)]
    res = run_bass_kernel_spmd(nc, in_maps, list(range(NCORES)), trace=trace)
    outs = []
    for c in range(NCORES):
        lt = res.results[c]["logitsT"]          # [BL, V, T]
        outs.append(lt.transpose(0, 2, 1))      # [BL, T, V]
    full = np.concatenate(outs, axis=0)         # [B, T, V]
    return full, res


def kernel(**inputs):
    out, _ = run_on_hw(inputs, trace=False)
    return out


# revision 11
# speedup vs baseline: 1.3492x; 1.1648x over previous
"""Trainium2 Bass kernel: GPT-style transformer forward pass.

Strategy: data-parallel over batch across 8 NeuronCores (B=64 -> 8 per core),
weights replicated.  On each core, activations are kept feature-major
(x_T [D=384 (3x128 partitions), 2048 tokens]) so matmuls contract the
partition dim with no activation transposes; LN/QKV/proj/FFN run on
512-token pairs of batch elements.  All matmul operands are bfloat16 (PSUM
accumulation stays fp32): on TRN2 hardware bf16 streams the PE at 2.4 GHz
while fp32r runs at the 1.2 GHz "others" clock, so bf16 doubles matmul
throughput on top of halving LDWEIGHTS and weight DMA.  The fp32 residual
stream x is kept in SBUF; a bf16 shadow is cast on the (otherwise idle)
GPSIMD engine for the LN mean matmuls.  LayerNorm stats come from
ones-vector matmuls (partition reduction); rstd = exp(-0.5*ln(var+eps)) on
the Scalar engine so the whole kernel lives in the natural_log_exp
activation table set.  Attention computes transposed scores S_T[s,t] per
(b,h) for the three live causal blocks only, exp on Scalar, one grouped
multiplicative mask per 3 heads on Vector, then token-major PV with an
appended ones-column in V so softmax denominators land as a per-partition
column, and PE transposes carry att back to feature-major for the
projection.  Q/K biases ride the PSUM->SBUF copy as tensor_scalar adds
instead of rank-1 matmuls.
"""

import os
import sys

for _p in ("/opt/trn_rl_repo",):
    if _p not in sys.path and os.path.isdir(_p):
        sys.path.insert(0, _p)

import numpy as np
import ml_dtypes

import concourse.bass as bass
import concourse.mybir as mybir
import concourse.tile as tile
from concourse import bacc
from concourse.bass_utils import run_bass_kernel_spmd

F32 = mybir.dt.float32
F32R = mybir.dt.float32r
BF16 = mybir.dt.bfloat16
NPBF = ml_dtypes.bfloat16
AF = mybir.ActivationFunctionType
OP = mybir.AluOpType

V, D, H, HD, L, T, B = 65, 384, 6, 64, 6, 256, 64
NCORES = 8
BL = B // NCORES          # batch elements per core
NTOK = BL * T             # tokens per core
DFF = 4 * D               # 1536
EPS = 1e-3
KC = D // 128             # 3 contraction chunks of 128
MT = D // 128             # 3 output feature tiles
FT = DFF // 128           # 12 ffn tiles
HD1 = HD + 2              # V columns incl ones col (64) + pad (65)
NBC = MT + FT + MT + MT + MT  # biasc cols: proj, ffn1, ffn2, q, k

MDT = BF16

# cst layout: cols 0:512 ones, col 512 = 1/D, cols 513:641 identity 128x128,
# cols 641:769 = 1/D block (stats matmul lhsT)
CST_W = 513 + 128 + 128


def _patch_act_tables():
    """Steer the activation-table picker to natural_log_exp_and_others for
    Exp and Ln, so this kernel's ACT stream never switches table sets."""
    if getattr(bacc, "_act_tables_patched", False):
        return
    real = bacc.get_activation_tables

    def patched(arch):
        t = real(arch)
        exp = mybir.ActivationFunctionType.Exp
        ln = mybir.ActivationFunctionType.Ln
        out = {}
        for name, fns in t.items():
            if name != "natural_log_exp_and_others":
                fns = fns - {exp, ln}
            out[name] = fns
        return out

    bacc.get_activation_tables = patched
    bacc._act_tables_patched = True


class _MM:
    """matmul emitter with explicit accumulation-chain boundaries."""

    def __init__(self, nc):
        self.nc = nc

    def __call__(self, out, lhsT, rhs, first=True, last=True, tile_position=None):
        self.nc.tensor.matmul(
            out, lhsT, rhs, start=first, stop=last, tile_position=tile_position,
        )


def build_program(n_layers=L, n_b=BL, n_heads=H):
    _patch_act_tables()
    assert n_b % 2 == 0 or n_b == 1
    ntok = n_b * T
    npair = max(1, n_b // 2)
    PW = 512 if n_b > 1 else 256      # tokens per pair-chunk
    nc = bacc.Bacc("TRN2", target_bir_lowering=False, debug=False)

    # ---------------- DRAM I/O ----------------
    hot_d = nc.dram_tensor("hotT", [V, ntok], MDT, kind="ExternalInput").ap()
    temb_d = nc.dram_tensor("temb", [V, D], MDT, kind="ExternalInput").ap()
    pos_d = nc.dram_tensor("posT", [128, KC, 512], MDT, kind="ExternalInput").ap()
    mask_d = nc.dram_tensor("mask", [128, 384], MDT, kind="ExternalInput").ap()
    wqkv_d = nc.dram_tensor("wqkv", [n_layers, 128, 3, KC, D], MDT, kind="ExternalInput").ap()
    wproj_d = nc.dram_tensor("wproj", [n_layers, 128, KC, D], MDT, kind="ExternalInput").ap()
    w1_d = nc.dram_tensor("w1", [n_layers, 128, KC, DFF], MDT, kind="ExternalInput").ap()
    w2_d = nc.dram_tensor("w2", [n_layers, 128, FT, D], MDT, kind="ExternalInput").ap()
    vbias_d = nc.dram_tensor("vbias", [n_layers, 128, D], F32, kind="ExternalInput").ap()
    biasc_d = nc.dram_tensor("biasc", [n_layers, 128, NBC], F32, kind="ExternalInput").ap()
    whead_d = nc.dram_tensor("whead", [128, KC, V], MDT, kind="ExternalInput").ap()
    bhead_d = nc.dram_tensor("bheadc", [V, 1], F32, kind="ExternalInput").ap()
    cst_d = nc.dram_tensor("cst", [128, CST_W], MDT, kind="ExternalInput").ap()
    cstr_d = nc.dram_tensor("cstr", [128, 128], F32R, kind="ExternalInput").ap()
    out_d = nc.dram_tensor("logitsT", [n_b, V, T], F32, kind="ExternalOutput").ap()

    from contextlib import ExitStack

    with tile.TileContext(nc) as tc, \
         nc.allow_low_precision(reason="bf16 matmul operand production"), \
         ExitStack() as ctx:
        ep = ctx.enter_context

        # ---------------- pools ----------------
        cpool = ep(tc.tile_pool(name="consts", bufs=1))
        xpool = ep(tc.tile_pool(name="x", bufs=1))
        wpool_qkv = ep(tc.tile_pool(name="wqkv", bufs=2))
        wpool_proj = ep(tc.tile_pool(name="wproj", bufs=2))
        wpool_1 = ep(tc.tile_pool(name="w1", bufs=2))
        wpool_2 = ep(tc.tile_pool(name="w2", bufs=2))
        wpool_b = ep(tc.tile_pool(name="wbias", bufs=2))
        hpool = ep(tc.tile_pool(name="h", bufs=3))
        xsqpool = ep(tc.tile_pool(name="xsq", bufs=2))
        qpool = ep(tc.tile_pool(name="q", bufs=2))
        kpool = ep(tc.tile_pool(name="k", bufs=2))
        vpool = ep(tc.tile_pool(name="v", bufs=1))
        upool = ep(tc.tile_pool(name="u", bufs=2))
        atmpool = ep(tc.tile_pool(name="atm", bufs=2))
        attpool = ep(tc.tile_pool(name="att", bufs=1))
        h1pool = ep(tc.tile_pool(name="h1", bufs=1))
        lgpool = ep(tc.tile_pool(name="lg", bufs=1))
        stpool = ep(tc.tile_pool(name="st", bufs=6))
        rdpool = ep(tc.tile_pool(name="rd", bufs=4))

        pbig = ep(tc.tile_pool(name="pbig", bufs=4, space="PSUM"))
        ppv = ep(tc.tile_pool(name="ppv", bufs=2, space="PSUM"))
        pstat = ep(tc.tile_pool(name="pstat", bufs=2, space="PSUM"))

        mm = _MM(nc)

        # ---------------- constants ----------------
        cst = cpool.tile([128, CST_W], MDT, name="cst_c")
        nc.sync.dma_start(out=cst[:, :], in_=cst_d[:, :])
        ones = cst[:, 0:512]
        ident = cst[:, 513:641]
        invD = cst[:, 641:769]
        mask = cpool.tile([128, 384], MDT, name="mask_c")
        nc.sync.dma_start(out=mask[:, :], in_=mask_d[:, :])
        invDr = cpool.tile([128, 128], F32R, name="cstr_c")
        nc.sync.dma_start(out=invDr[:, :], in_=cstr_d[:, :])
        whead = cpool.tile([128, KC, V], MDT, name="whead_c")
        nc.sync.dma_start(out=whead[:, :, :], in_=whead_d[:, :, :])
        bhead = cpool.tile([V, 1], F32, name="bhead_c")
        nc.sync.dma_start(out=bhead[:, :], in_=bhead_d[:, :])

        x = xpool.tile([128, KC, ntok], F32R, name="x_resid")

        nbp = PW // T
        v_tiles = [vpool.tile([128, 2 * nbp, n_heads, HD1], MDT, name=f"v_pp{i}")
                   for i in range(2)]
        for vt in v_tiles:
            for tb in range(2 * nbp):
                nc.vector.tensor_copy(vt[:, tb, :, HD:HD1],
                                      ones[:, 0:2 * n_heads].rearrange(
                                          "p (h c) -> p h c", h=n_heads))

        # ---------------- embedding (scratch tiles borrow other pools) ----------------
        pos = attpool.tile([128, KC, 512], MDT, tag="att")
        nc.sync.dma_start(out=pos[:, :, :], in_=pos_d[:, :, :])
        temb = stpool.tile([V, 384], MDT, tag="st")
        nc.sync.dma_start(out=temb[:, :], in_=temb_d[:, :])
        for ch in range(ntok // PW):
            cs = slice(ch * PW, ch * PW + PW)
            hot = xsqpool.tile([V, PW], MDT, tag="xsq")
            nc.sync.dma_start(out=hot[:, :], in_=hot_d[:, cs])
            for c in range(KC):
                ps = pbig.tile([128, 512], F32, tag="pbig")
                mm(ps[:, 0:PW], temb[0:V, c * 128:(c + 1) * 128], hot[0:V, :])
                nc.vector.tensor_tensor(x[:, c, cs], ps[:, 0:PW], pos[:, c, 0:PW], op=OP.add)

        # ---------------- LN split into stats + finish ----------------
        def ln_stats(p):
            """square (ACT) + M=128 stat matmuls (mean/meansq arrive
            pre-broadcast across partitions; mean feeds only the variance —
            the apply-side mean subtraction is folded into centered weights)
            + the DVE/ACT rstd chain.  The mean chain reads the fp32r
            residual directly so nothing casts x on the critical path."""
            pc = slice(p * PW, p * PW + PW)
            xsq = xsqpool.tile([128, KC, PW], MDT, tag="xsq")
            nc.scalar.activation(xsq[:, :, :], x[:, :, pc], AF.Square)
            meanb = pstat.tile([128, PW], F32, tag="pstat")
            msqb = pstat.tile([128, PW], F32, tag="pstat")
            for c in range(KC):
                mm(meanb[:, :], invDr[:, :], x[:, c, pc],
                   first=(c == 0), last=(c == KC - 1))
            for c in range(KC):
                mm(msqb[:, :], invD[:, :], xsq[:, c, :],
                   first=(c == 0), last=(c == KC - 1))
            m2b = stpool.tile([128, PW], F32, tag="st")
            nc.scalar.activation(m2b[:, :], meanb[:, :], AF.Square)
            varb = stpool.tile([128, PW], F32, tag="st")
            nc.vector.scalar_tensor_tensor(
                varb[:, :], msqb[:, :], EPS, m2b[:, :],
                op0=OP.add, op1=OP.subtract)
            nc.scalar.activation(varb[:, :], varb[:, :], AF.Ln)
            rstdb = stpool.tile([128, PW], F32, tag="st")
            nc.scalar.activation(rstdb[:, :], varb[:, :], AF.Exp, scale=-0.5)
            return (p, rstdb)

        def ln_finish(tok):
            """apply -> h (one DVE op; mean handled by centered weights)"""
            p, rstdb = tok
            pc = slice(p * PW, p * PW + PW)
            h = hpool.tile([128, KC, PW], MDT, tag="h")
            nc.vector.tensor_tensor(
                h[:, :, :], x[:, :, pc],
                rstdb[:, None, :].broadcast_to([128, KC, PW]), op=OP.mult)
            return h

        def run(mids):
            out = []
            for f in mids:
                out.append(f())
            return out

        # ---------------- phase B: qkv + attention + proj for a pair ----------------
        def emit_B(p, h, wqkv, wproj, vbias, biasc, mid_a=(), mid_b=()):
            pc = slice(p * PW, p * PW + PW)
            nb_in_p = PW // T
            q_t = qpool.tile([128, MT, PW], MDT, tag="q")
            k_t = kpool.tile([128, MT, PW], MDT, tag="k")
            for mat, dst, bcol in ((0, q_t, MT + FT + MT), (1, k_t, MT + FT + 2 * MT)):
                for mt in range(MT):
                    ps = pbig.tile([128, 512], F32, tag="pbig")
                    for kc in range(KC):
                        mm(ps[:, 0:PW], wqkv[:, mat, kc, mt * 128:(mt + 1) * 128],
                           h[:, kc, :], first=(kc == 0), last=(kc == KC - 1))
                    nc.vector.tensor_scalar(
                        dst[:, mt, :], ps[:, 0:PW],
                        biasc[:, bcol + mt:bcol + mt + 1], None, op0=OP.add)
            v_t = v_tiles[p % 2]
            for tb in range(2 * nb_in_p):
                vps = pbig.tile([128, 512], F32, tag="pbig")
                for kc in range(KC):
                    mm(vps[:, 0:D], h[:, kc, tb * 128:(tb + 1) * 128],
                       wqkv[:, 2, kc, :], first=(kc == 0), last=(kc == KC - 1))
                nc.vector.tensor_tensor(
                    v_t[:, tb, :, 0:HD],
                    vps[:, 0:D].rearrange("p (h d) -> p h d", h=n_heads),
                    vbias[:, :].rearrange("p (h d) -> p h d", h=n_heads),
                    op=OP.add)
            mids_out = run(mid_a)
            atms = []
            for bi in range(nb_in_p):
                boff = bi * T
                if bi == 1:
                    mids_out += run(mid_b)
                us = upool.tile([128, n_heads, 384], MDT, tag="u")
                pv0 = ppv.tile([128, n_heads, HD1], F32, tag="ppv")
                pv1 = ppv.tile([128, n_heads, HD1], F32, tag="ppv")

                def emit_S(hh):
                    # us[:, hh, 0:256]  = S_T[s in chunk0, t 0:256]
                    # us[:, hh, 256:384] = S_T[s in chunk1, t 128:256]
                    hp = 64 * (hh % 2)
                    hc = hh // 2
                    sps = pbig.tile([128, 384], F32, tag="pbig")
                    mm(sps[:, 0:256], k_t[hp:hp + HD, hc, boff:boff + 128],
                       q_t[hp:hp + HD, hc, boff:boff + T])
                    mm(sps[:, 256:384], k_t[hp:hp + HD, hc, boff + 128:boff + 256],
                       q_t[hp:hp + HD, hc, boff + 128:boff + 256])
                    nc.scalar.activation(us[:, hh, :], sps[:, :], AF.Exp)

                def emit_mask(h0):
                    # zero the masked upper-triangles of the two diagonal
                    # blocks (cols 0:128 and 256:384); cols 128:256 are the
                    # fully-live (s chunk0, t 128:256) block (mask==1 there)
                    nc.vector.tensor_tensor(
                        us[:, h0:h0 + 3, :], us[:, h0:h0 + 3, :],
                        mask[:, None, :].broadcast_to([128, 3, 384]), op=OP.mult)

                def emit_PV(hh):
                    mm(pv0[:, hh, :], us[:, hh, 0:128], v_t[:, 2 * bi, hh, :])
                    mm(pv1[:, hh, :], us[:, hh, 128:256], v_t[:, 2 * bi, hh, :],
                       first=True, last=False)
                    mm(pv1[:, hh, :], us[:, hh, 256:384], v_t[:, 2 * bi + 1, hh, :],
                       first=False, last=True)

                emit_S(0); emit_S(1); emit_S(2)
                emit_mask(0)
                emit_PV(0); emit_S(3)
                emit_PV(1); emit_S(4)
                emit_PV(2); emit_S(5)
                emit_mask(3)
                emit_PV(3); emit_PV(4); emit_PV(5)
                atm = atmpool.tile([128, 2, n_heads * HD], MDT, tag="atm")
                for tb, pv in ((0, pv0), (1, pv1)):
                    rden = rdpool.tile([128, n_heads], F32, tag="rd")
                    nc.vector.reciprocal(rden[:, :], pv[:, :, HD])
                    nc.vector.tensor_tensor(
                        atm[:, tb, :].rearrange("p (h d) -> p h d", h=n_heads),
                        pv[:, :, 0:HD],
                        rden[:, :, None].broadcast_to([128, n_heads, HD]),
                        op=OP.mult)
                atms.append(atm)
            att_t = attpool.tile([128, KC, PW], MDT, tag="att")
            for c in range(KC):
                tps = pbig.tile([128, 512], MDT, tag="pbig")
                for bi in range(nb_in_p):
                    for tb in range(2):
                        col = (bi * 2 + tb) * 128
                        nc.tensor.transpose(
                            tps[:, col:col + 128],
                            atms[bi][:, tb, c * 128:(c + 1) * 128],
                            ident[:, :])
                nc.vector.tensor_copy(att_t[:, c, :], tps[:, 0:PW])
            for mt in range(MT):
                pp = pbig.tile([128, 512], F32, tag="pbig")
                for kc in range(KC):
                    mm(pp[:, 0:PW], wproj[:, kc, mt * 128:(mt + 1) * 128],
                       att_t[:, kc, :], first=(kc == 0), last=(kc == KC - 1))
                nc.vector.scalar_tensor_tensor(
                    x[:, mt, pc], pp[:, 0:PW], biasc[:, mt:mt + 1], x[:, mt, pc],
                    op0=OP.add, op1=OP.add)
            return mids_out

        # ---------------- phase D: FFN for a pair ----------------
        def emit_D(p, h2, w1, w2, biasc, mid=()):
            pc = slice(p * PW, p * PW + PW)
            h1_t = h1pool.tile([128, FT, PW], MDT, tag="h1")
            for mt in range(FT):
                fps = pbig.tile([128, 512], F32, tag="pbig")
                for kc in range(KC):
                    mm(fps[:, 0:PW], w1[:, kc, mt * 128:(mt + 1) * 128],
                       h2[:, kc, :], first=(kc == 0), last=(kc == KC - 1))
                nc.scalar.activation(h1_t[:, mt, :], fps[:, 0:PW], AF.Relu,
                                     bias=biasc[:, MT + mt:MT + mt + 1])
            mids_out = run(mid)
            for mt in range(MT):
                fp2 = pbig.tile([128, 512], F32, tag="pbig")
                for kc in range(FT):
                    mm(fp2[:, 0:PW], w2[:, kc, mt * 128:(mt + 1) * 128],
                       h1_t[:, kc, :], first=(kc == 0), last=(kc == FT - 1))
                nc.vector.scalar_tensor_tensor(
                    x[:, mt, pc], fp2[:, 0:PW],
                    biasc[:, MT + FT + mt:MT + FT + mt + 1],
                    x[:, mt, pc], op0=OP.add, op1=OP.add)
            return mids_out

        # ---------------- layers: software-pipelined emission ----------------
        carry = {}
        for l in range(n_layers):
            wqkv = wpool_qkv.tile([128, 3, KC, D], MDT, tag="wqkv")
            nc.sync.dma_start(out=wqkv[:, :, :, :], in_=wqkv_d[l])
            wproj = wpool_proj.tile([128, KC, D], MDT, tag="wproj")
            nc.sync.dma_start(out=wproj[:, :, :], in_=wproj_d[l])
            w1 = wpool_1.tile([128, KC, DFF], MDT, tag="w1")
            nc.sync.dma_start(out=w1[:, :, :], in_=w1_d[l])
            w2 = wpool_2.tile([128, FT, D], MDT, tag="w2")
            nc.sync.dma_start(out=w2[:, :, :], in_=w2_d[l])
            vbias = wpool_b.tile([128, D], F32, tag="vbias")
            nc.sync.dma_start(out=vbias[:, :], in_=vbias_d[l])
            biasc = wpool_b.tile([128, NBC], F32, tag="biasc")
            nc.sync.dma_start(out=biasc[:, :], in_=biasc_d[l])

            Bp = lambda p, h, **kw: emit_B(p, h, wqkv, wproj, vbias, biasc, **kw)
            Dp = lambda p, h2, **kw: emit_D(p, h2, w1, w2, biasc, **kw)

            if npair == 4:
                if l == 0:
                    h0 = ln_finish(ln_stats(0))
                    s1 = ln_stats(1)
                else:
                    h0, s1 = carry["h0"], carry["s1"]
                (h1,) = Bp(0, h0, mid_a=[lambda: ln_finish(s1)])
                sc0 = ln_stats(0)
                h2_0, s2 = Bp(1, h1, mid_a=[lambda: ln_finish(sc0),
                                            lambda: ln_stats(2)])
                (g2,) = Dp(0, h2_0, mid=[lambda: ln_finish(s2)])
                sc1 = ln_stats(1)
                h2_1, s3 = Bp(2, g2, mid_a=[lambda: ln_finish(sc1),
                                            lambda: ln_stats(3)])
                (g3,) = Dp(1, h2_1, mid=[lambda: ln_finish(s3)])
                sc2 = ln_stats(2)
                (h2_2,) = Bp(3, g3, mid_a=[lambda: ln_finish(sc2)])
                last = (l == n_layers - 1)
                if not last:
                    sc3, s0n = Dp(2, h2_2, mid=[lambda: ln_stats(3),
                                                lambda: ln_stats(0)])
                    # note: ln_stats(0) here reads x pair0 as updated by D0 above
                    h2_3 = ln_finish(sc3)
                    (h0n,) = Dp(3, h2_3, mid=[lambda: ln_finish(s0n)])
                    carry = {"h0": h0n, "s1": ln_stats(1)}
                else:
                    (sc3,) = Dp(2, h2_2, mid=[lambda: ln_stats(3)])
                    h2_3 = ln_finish(sc3)
                    Dp(3, h2_3)
            else:
                # simple order for small test configs
                hq = {}
                hq[0] = ln_finish(ln_stats(0))
                for p in range(1, npair):
                    hq[p] = ln_finish(ln_stats(p))
                    Bp(p - 1, hq.pop(p - 1))
                Bp(npair - 1, hq.pop(npair - 1))
                hq[0] = ln_finish(ln_stats(0))
                for p in range(1, npair):
                    hq[p] = ln_finish(ln_stats(p))
                    Dp(p - 1, hq.pop(p - 1))
                Dp(npair - 1, hq.pop(npair - 1))

        # ---------------- final LN + head ----------------
        for p in range(npair):
            hf = ln_finish(ln_stats(p))
            for bi in range(PW // T):
                b = p * (PW // T) + bi
                hps = ppv.tile([V, 256], F32, tag="ppv")
                for kc in range(KC):
                    mm(hps[:, :], whead[:, kc, :], hf[:, kc, bi * T:(bi + 1) * T],
                       first=(kc == 0), last=(kc == KC - 1))
                lg = lgpool.tile([V, T], F32, tag="lg")
                nc.vector.tensor_scalar(lg[:, :], hps[:, :], bhead[0:V, 0:1], None,
                                        op0=OP.add)
                nc.sync.dma_start(out=out_d[b], in_=lg[:, :])

    nc.compile()
    return nc


# ---------------------------------------------------------------------------
# host side
# ---------------------------------------------------------------------------

def prep_inputs(inputs, n_layers=L, n_b=BL, core=0):
    """Build the per-core input map (numpy) for `core`."""
    f32 = np.float32
    idx = np.asarray(inputs["idx"])
    tok_emb = np.asarray(inputs["tok_emb"], f32)
    pos_emb = np.asarray(inputs["pos_emb"], f32)
    Wq = np.asarray(inputs["Wq"], f32)
    Wk = np.asarray(inputs["Wk"], f32)
    Wv = np.asarray(inputs["Wv"], f32)
    Wproj = np.asarray(inputs["Wproj"], f32)
    bproj = np.asarray(inputs["bproj"], f32)
    W1 = np.asarray(inputs["W1"], f32)
    b1 = np.asarray(inputs["b1"], f32)
    W2 = np.asarray(inputs["W2"], f32)
    b2 = np.asarray(inputs["b2"], f32)
    ln1_g = np.asarray(inputs["ln1_g"], f32)
    ln1_b = np.asarray(inputs["ln1_b"], f32)
    ln2_g = np.asarray(inputs["ln2_g"], f32)
    ln2_b = np.asarray(inputs["ln2_b"], f32)
    lnf_g = np.asarray(inputs["lnf_g"], f32)
    lnf_b = np.asarray(inputs["lnf_b"], f32)
    Whead = np.asarray(inputs["Whead"], f32)
    bhead = np.asarray(inputs["bhead"], f32)

    ntok = n_b * T
    scale = f32(D) ** -0.5

    idx_c = idx[core * n_b:(core + 1) * n_b].reshape(-1)         # [ntok]
    hot = (idx_c[None, :] == np.arange(V)[:, None]).astype(f32)  # [V, ntok]

    posT = pos_emb.T.astype(f32)                                 # [D, T]
    posT2 = np.concatenate([posT, posT], axis=1)                 # [D, 512]
    pos_in = posT2.reshape(KC, 128, 512).transpose(1, 0, 2).copy()

    lane = np.arange(128)
    t = np.arange(T)
    tri0 = (lane[:, None] <= t[None, :128]).astype(f32)          # diag block
    mask = np.concatenate([tri0, np.ones((128, 128), f32), tri0], axis=1)

    def pack_w(w):  # [D_in, N] -> [128, KC_in, N]
        kin = w.shape[0] // 128
        return w.reshape(kin, 128, -1).transpose(1, 0, 2).copy()

    wqkv = np.zeros((n_layers, 128, 3, KC, D), f32)
    wproj = np.zeros((n_layers, 128, KC, D), f32)
    w1 = np.zeros((n_layers, 128, KC, DFF), f32)
    w2 = np.zeros((n_layers, 128, FT, D), f32)
    vbias = np.zeros((n_layers, 128, D), f32)
    biasc = np.zeros((n_layers, 128, NBC), f32)

    # LN mean subtraction is folded into every weight that consumes a
    # post-LN activation: W' = center(g * W) since (x - mean(x)) @ Wg
    # == x @ (Wg - colmean(Wg)) for any per-token scaling of x.
    center = lambda w: w - w.mean(axis=0, keepdims=True)

    for l in range(n_layers):
        # Wq[l] is [H, D, HD]; feature f = h*HD+hd -> transpose to [D, H, HD]
        wq2 = Wq[l].transpose(1, 0, 2).reshape(D, D) * scale
        wk2 = Wk[l].transpose(1, 0, 2).reshape(D, D)
        wv2 = Wv[l].transpose(1, 0, 2).reshape(D, D)
        wqkv[l, :, 0] = pack_w(center(wq2 * ln1_g[l][:, None]))
        wqkv[l, :, 1] = pack_w(center(wk2 * ln1_g[l][:, None]))
        wqkv[l, :, 2] = pack_w(center(wv2 * ln1_g[l][:, None]))
        vbias[l] = np.broadcast_to(ln1_b[l] @ wv2, (128, D))
        wproj[l] = pack_w(Wproj[l])
        w1[l] = pack_w(center(W1[l] * ln2_g[l][:, None]))
        w2[l] = pack_w(W2[l])
        biasc[l, :, 0:MT] = bproj[l].reshape(MT, 128).T
        biasc[l, :, MT:MT + FT] = (b1[l] + ln2_b[l] @ W1[l]).reshape(FT, 128).T
        biasc[l, :, MT + FT:MT + FT + MT] = b2[l].reshape(MT, 128).T
        biasc[l, :, MT + FT + MT:MT + FT + 2 * MT] = \
            (ln1_b[l] @ wq2).reshape(MT, 128).T
        biasc[l, :, MT + FT + 2 * MT:] = (ln1_b[l] @ wk2).reshape(MT, 128).T

    whead_eff = center(Whead * lnf_g[:, None])
    bhead_eff = (bhead + lnf_b @ Whead).astype(f32)

    cst = np.ones((128, CST_W), f32)
    cst[:, 512] = 1.0 / D
    cst[:, 513:641] = np.eye(128, dtype=f32)
    cst[:, 641:769] = 1.0 / D

    bf = lambda a: np.ascontiguousarray(a).astype(NPBF)

    return {
        "cst": bf(cst),
        "cstr": np.full((128, 128), 1.0 / D, f32),
        "hotT": bf(hot),
        "temb": bf(tok_emb),
        "posT": bf(pos_in),
        "mask": bf(mask),
        "wqkv": bf(wqkv),
        "wproj": bf(wproj),
        "w1": bf(w1),
        "w2": bf(w2),
        "vbias": vbias,
        "biasc": biasc,
        "whead": bf(pack_w(whead_eff)),
        "bheadc": bhead_eff[:, None].copy(),
    }


_CACHE = {}


def get_program():
    if "nc" not in _CACHE:
        _CACHE["nc"] = build_program()
    return _CACHE["nc"]


def run_on_hw(inputs, trace=False):
    nc = get_program()
    in_maps = [prep_inputs(inputs, core=c) for c in range(NCORES)]
    res = run_bass_kernel_spmd(nc, in_maps, list(range(NCORES)), trace=trace)
    outs = []
    for c in range(NCORES):
        lt = res.results[c]["logitsT"]          # [BL, V, T]
        outs.append(lt.transpose(0, 2, 1))      # [BL, T, V]
    full = np.concatenate(outs, axis=0)         # [B, T, V]
    return full, res


def kernel(**inputs):
    out, _ = run_on_hw(inputs, trace=False)
    return out


# revision 38
# speedup vs baseline: 1.4662x; 1.0867x over previous
"""Trainium2 Bass kernel: GPT-style transformer forward pass.

Strategy: data-parallel over batch across 8 NeuronCores (B=64 -> 8 per core),
weights replicated.  On each core, activations are kept feature-major
(x_T [D=384 (3x128 partitions), 2048 tokens]) so matmuls contract the
partition dim with no activation transposes; LN/QKV/proj/FFN run on
512-token pairs of batch elements.  All matmul operands are bfloat16 (PSUM
accumulation stays fp32): on TRN2 hardware bf16 streams the PE at 2.4 GHz
while fp32r runs at the 1.2 GHz "others" clock, so bf16 doubles matmul
throughput on top of halving LDWEIGHTS and weight DMA.  The fp32 residual
stream x is kept in SBUF; a bf16 shadow is cast on the (otherwise idle)
GPSIMD engine for the LN mean matmuls.  LayerNorm stats come from
ones-vector matmuls (partition reduction); rstd = exp(-0.5*ln(var+eps)) on
the Scalar engine so the whole kernel lives in the natural_log_exp
activation table set.  Attention computes transposed scores S_T[s,t] per
(b,h) for the three live causal blocks only, exp on Scalar, one grouped
multiplicative mask per 3 heads on Vector, then token-major PV with an
appended ones-column in V so softmax denominators land as a per-partition
column, and PE transposes carry att back to feature-major for the
projection.  Q/K biases ride the PSUM->SBUF copy as tensor_scalar adds
instead of rank-1 matmuls.
"""

import os
import sys

for _p in ("/opt/trn_rl_repo",):
    if _p not in sys.path and os.path.isdir(_p):
        sys.path.insert(0, _p)

import numpy as np
import ml_dtypes

import concourse.bass as bass
import concourse.mybir as mybir
import concourse.tile as tile
from concourse import bacc
from concourse.bass_utils import run_bass_kernel_spmd

F32 = mybir.dt.float32
F32R = mybir.dt.float32r
BF16 = mybir.dt.bfloat16
NPBF = ml_dtypes.bfloat16
AF = mybir.ActivationFunctionType
OP = mybir.AluOpType

V, D, H, HD, L, T, B = 65, 384, 6, 64, 6, 256, 64
NCORES = 8
BL = B // NCORES          # batch elements per core
NTOK = BL * T             # tokens per core
DFF = 4 * D               # 1536
EPS = 1e-3
KC = D // 128             # 3 contraction chunks of 128
MT = D // 128             # 3 output feature tiles
FT = DFF // 128           # 12 ffn tiles
HD1 = HD + 2              # V columns incl ones col (64) + pad (65)
NBC = MT + FT + MT + MT + MT  # biasc cols: proj, ffn1, ffn2, q, k

MDT = BF16

# cst layout: cols 0:512 ones, col 512 = 1/D, cols 513:641 identity 128x128,
# cols 641:769 = 1/D block (stats matmul lhsT)
CST_W = 513 + 128 + 128


def _patch_act_tables():
    """Steer the activation-table picker to natural_log_exp_and_others for
    Exp and Ln, so this kernel's ACT stream never switches table sets."""
    if getattr(bacc, "_act_tables_patched", False):
        return
    real = bacc.get_activation_tables

    def patched(arch):
        t = real(arch)
        exp = mybir.ActivationFunctionType.Exp
        ln = mybir.ActivationFunctionType.Ln
        out = {}
        for name, fns in t.items():
            if name != "natural_log_exp_and_others":
                fns = fns - {exp, ln}
            out[name] = fns
        return out

    bacc.get_activation_tables = patched
    bacc._act_tables_patched = True


class _MM:
    """matmul emitter with explicit accumulation-chain boundaries."""

    def __init__(self, nc):
        self.nc = nc

    def __call__(self, out, lhsT, rhs, first=True, last=True, tile_position=None):
        self.nc.tensor.matmul(
            out, lhsT, rhs, start=first, stop=last, tile_position=tile_position,
        )


def build_program(n_layers=L, n_b=BL, n_heads=H):
    _patch_act_tables()
    assert n_b % 2 == 0 or n_b == 1
    ntok = n_b * T
    npair = max(1, n_b // 2)
    PW = 512 if n_b > 1 else 256      # tokens per pair-chunk
    nc = bacc.Bacc("TRN2", target_bir_lowering=False, debug=False)

    # ---------------- DRAM I/O ----------------
    hot_d = nc.dram_tensor("hotT", [V, ntok], MDT, kind="ExternalInput").ap()
    temb_d = nc.dram_tensor("temb", [V, D], MDT, kind="ExternalInput").ap()
    pos_d = nc.dram_tensor("posT", [128, KC, 512], MDT, kind="ExternalInput").ap()
    mask_d = nc.dram_tensor("mask", [128, 384], MDT, kind="ExternalInput").ap()
    wqkv_d = nc.dram_tensor("wqkv", [n_layers, 128, 3, KC, D], MDT, kind="ExternalInput").ap()
    wproj_d = nc.dram_tensor("wproj", [n_layers, 128, KC, D], MDT, kind="ExternalInput").ap()
    w1_d = nc.dram_tensor("w1", [n_layers, 128, KC, DFF], MDT, kind="ExternalInput").ap()
    w2_d = nc.dram_tensor("w2", [n_layers, 128, FT, D], MDT, kind="ExternalInput").ap()
    vbias_d = nc.dram_tensor("vbias", [n_layers, 128, D], F32, kind="ExternalInput").ap()
    biasc_d = nc.dram_tensor("biasc", [n_layers, 128, NBC], F32, kind="ExternalInput").ap()
    whead_d = nc.dram_tensor("whead", [128, KC, V], MDT, kind="ExternalInput").ap()
    bhead_d = nc.dram_tensor("bheadc", [V, 1], F32, kind="ExternalInput").ap()
    cst_d = nc.dram_tensor("cst", [128, CST_W], MDT, kind="ExternalInput").ap()
    epsc_d = nc.dram_tensor("epsc", [128, 1], F32, kind="ExternalInput").ap()
    out_d = nc.dram_tensor("logitsT", [n_b, V, T], F32, kind="ExternalOutput").ap()

    from contextlib import ExitStack

    with tile.TileContext(nc) as tc, \
         nc.allow_low_precision(reason="bf16 matmul operand production"), \
         ExitStack() as ctx:
        ep = ctx.enter_context

        # ---------------- pools ----------------
        cpool = ep(tc.tile_pool(name="consts", bufs=1))
        xpool = ep(tc.tile_pool(name="x", bufs=1))
        wpool_qkv = ep(tc.tile_pool(name="wqkv", bufs=2))
        wpool_proj = ep(tc.tile_pool(name="wproj", bufs=2))
        wpool_1 = ep(tc.tile_pool(name="w1", bufs=2))
        wpool_2 = ep(tc.tile_pool(name="w2", bufs=2))
        wpool_b = ep(tc.tile_pool(name="wbias", bufs=2))
        hpool = ep(tc.tile_pool(name="h", bufs=3))
        xsqpool = ep(tc.tile_pool(name="xsq", bufs=2))
        qpool = ep(tc.tile_pool(name="q", bufs=2))
        kpool = ep(tc.tile_pool(name="k", bufs=2))
        vpool = ep(tc.tile_pool(name="v", bufs=1))
        upool = ep(tc.tile_pool(name="u", bufs=2))
        atmpool = ep(tc.tile_pool(name="atm", bufs=2))
        attpool = ep(tc.tile_pool(name="att", bufs=1))
        h1pool = ep(tc.tile_pool(name="h1", bufs=2))
        lgpool = ep(tc.tile_pool(name="lg", bufs=1))
        stpool = ep(tc.tile_pool(name="st", bufs=6))
        rdpool = ep(tc.tile_pool(name="rd", bufs=4))

        pbig = ep(tc.tile_pool(name="pbig", bufs=5, space="PSUM"))
        ppv = ep(tc.tile_pool(name="ppv", bufs=2, space="PSUM"))
        pstat = ep(tc.tile_pool(name="pstat", bufs=1, space="PSUM"))

        mm = _MM(nc)

        # ---------------- constants (embedding-critical DMAs first) ----------------
        temb = stpool.tile([V, 384], MDT, tag="st")
        nc.sync.dma_start(out=temb[:, :], in_=temb_d[:, :])
        hots = []
        for ch in range(ntok // PW):
            hot = xsqpool.tile([V, PW], MDT, tag="xsq")
            nc.sync.dma_start(out=hot[:, :], in_=hot_d[:, ch * PW:(ch + 1) * PW])
            hots.append(hot)
        pos = attpool.tile([128, KC, 512], MDT, tag="att")
        nc.sync.dma_start(out=pos[:, :, :], in_=pos_d[:, :, :])
        cst = cpool.tile([128, CST_W], MDT, name="cst_c")
        nc.sync.dma_start(out=cst[:, :], in_=cst_d[:, :])
        ones = cst[:, 0:512]
        ident = cst[:, 513:641]
        invD = cst[:, 641:769]
        epsc = cpool.tile([128, 1], F32, name="epsc_c")
        nc.sync.dma_start(out=epsc[:, :], in_=epsc_d[:, :])
        mask = cpool.tile([128, 384], MDT, name="mask_c")
        nc.sync.dma_start(out=mask[:, :], in_=mask_d[:, :])
        whead = cpool.tile([128, KC, V], MDT, name="whead_c")
        nc.sync.dma_start(out=whead[:, :, :], in_=whead_d[:, :, :])
        bhead = cpool.tile([V, 1], F32, name="bhead_c")
        nc.sync.dma_start(out=bhead[:, :], in_=bhead_d[:, :])

        x = xpool.tile([128, KC, ntok], F32R, name="x_resid")

        nbp = PW // T
        v_tiles = [vpool.tile([128, 2 * nbp, n_heads, HD1], MDT, name=f"v_pp{i}")
                   for i in range(2)]
        for vt in v_tiles:
            for tb in range(2 * nbp):
                nc.vector.tensor_copy(vt[:, tb, :, HD:HD1],
                                      ones[:, 0:2 * n_heads].rearrange(
                                          "p (h c) -> p h c", h=n_heads))

        # ---------------- embedding ----------------
        for ch in range(ntok // PW):
            cs = slice(ch * PW, ch * PW + PW)
            hot = hots[ch]
            for c in range(KC):
                ps = pbig.tile([128, 512], F32, tag="pbig")
                mm(ps[:, 0:PW], temb[0:V, c * 128:(c + 1) * 128], hot[0:V, :])
                nc.vector.tensor_tensor(x[:, c, cs], ps[:, 0:PW], pos[:, c, 0:PW], op=OP.add)

        # ---------------- LN split into stats + finish ----------------
        def ln_stats(p):
            """square (ACT) + meansq matmuls (arrive pre-broadcast across
            partitions) + Ln/Exp rstd chain.  Every residual-stream
            contribution is column-centered host-side, so the feature-mean
            of x is identically zero: no mean chain, var = E[x^2]."""
            pc = slice(p * PW, p * PW + PW)
            xsq = xsqpool.tile([128, KC, PW], MDT, tag="xsq")
            nc.scalar.activation(xsq[:, :, :], x[:, :, pc], AF.Square)
            msqb = pstat.tile([128, PW], F32, tag="pstat")
            for c in range(KC):
                mm(msqb[:, :], invD[:, :], xsq[:, c, :],
                   first=(c == 0), last=(c == KC - 1))
            # invD is 1/512 (exact in bf16); rescale by 512/D inside the Ln
            varb = stpool.tile([128, PW], F32, tag="st")
            nc.scalar.activation(varb[:, :], msqb[:, :], AF.Ln,
                                 bias=epsc[:, 0:1], scale=512.0 / D)
            rstdb = stpool.tile([128, PW], F32, tag="st")
            nc.scalar.activation(rstdb[:, :], varb[:, :], AF.Exp, scale=-0.5)
            return (p, rstdb)

        def ln_finish(tok):
            """apply -> h (one DVE op; mean handled by centered weights)"""
            p, rstdb = tok
            pc = slice(p * PW, p * PW + PW)
            h = hpool.tile([128, KC, PW], MDT, tag="h")
            nc.vector.tensor_tensor(
                h[:, :, :], x[:, :, pc],
                rstdb[:, None, :].broadcast_to([128, KC, PW]), op=OP.mult)
            return h

        def run(mids):
            out = []
            for f in mids:
                out.append(f())
            return out

        # ---------------- FFN as a weavable unit stream ----------------
        class DWeave:
            """FFN work for one pair, consumable in-order as filler units.
            pad_mm() emits one ffn1 3-matmul chain (pure PE) and defers its
            relu; pad_relu() flushes deferred relus onto DVE (so the ACT exp
            stream in the attention phase is never delayed); drain() finishes
            everything (remaining relus on ACT)."""

            def __init__(self, p, h2, w1, w2, biasc, mid=()):
                self.p = p
                self.pc = slice(p * PW, p * PW + PW)
                self.h2 = h2
                self.w1, self.w2, self.biasc = w1, w2, biasc
                self.mid = mid
                self.mids = []
                self.h1_t = h1pool.tile([128, FT, PW], MDT, tag="h1")
                self.fps = {}
                self.i1 = 0          # next ffn1 chain to emit
                self.pending = []    # ffn1 tiles awaiting relu
                self.mid_done = False
                self.i2 = 0          # next ffn2 chain to emit

            def _chain1(self, mt):
                fps = pbig.tile([128, 512], F32, tag="pbig")
                for kc in range(KC):
                    mm(fps[:, 0:PW], self.w1[:, kc, mt * 128:(mt + 1) * 128],
                       self.h2[:, kc, :], first=(kc == 0), last=(kc == KC - 1))
                return fps

            def _relu(self, mt, fps, dve):
                bcol = self.biasc[:, MT + mt:MT + mt + 1]
                if dve:
                    nc.vector.tensor_scalar(
                        self.h1_t[:, mt, :], fps[:, 0:PW], bcol, 0.0,
                        op0=OP.add, op1=OP.max)
                else:
                    nc.scalar.activation(self.h1_t[:, mt, :], fps[:, 0:PW],
                                         AF.Relu, bias=bcol)

            def pad_mm(self):
                if self.i1 < FT and len(self.pending) < 2:
                    mt = self.i1
                    self.i1 += 1
                    self.pending.append((mt, self._chain1(mt)))

            def pad_relu(self):
                for mt, fps in self.pending:
                    self._relu(mt, fps, dve=True)
                self.pending = []

            def _chain2(self, mt):
                fp2 = pbig.tile([128, 512], F32, tag="pbig")
                for kc in range(FT):
                    mm(fp2[:, 0:PW], self.w2[:, kc, mt * 128:(mt + 1) * 128],
                       self.h1_t[:, kc, :], first=(kc == 0), last=(kc == FT - 1))
                nc.vector.scalar_tensor_tensor(
                    x[:, mt, self.pc], fp2[:, 0:PW],
                    self.biasc[:, MT + FT + mt:MT + FT + mt + 1],
                    x[:, mt, self.pc], op0=OP.add, op1=OP.add)

            def drain(self, n_f1=FT, f2=True):
                for mt, fps in self.pending:
                    self._relu(mt, fps, dve=True)
                self.pending = []
                while self.i1 < n_f1:
                    mt = self.i1
                    self.i1 += 1
                    self._relu(mt, self._chain1(mt), dve=False)
                if self.i1 < FT:
                    return
                if not self.mid_done:
                    self.mid_done = True
                    self.mids = run(self.mid)
                if f2:
                    while self.i2 < MT:
                        self._chain2(self.i2)
                        self.i2 += 1

        # ---------------- phase B: qkv + attention + proj for a pair ----------------
        def emit_B(p, h, wqkv, wproj, vbias, biasc, mid_a=(), fill=None,
                   fill_factory=None):
            pc = slice(p * PW, p * PW + PW)
            nb_in_p = PW // T
            q_t = qpool.tile([128, MT, PW], MDT, tag="q")
            k_t = kpool.tile([128, MT, PW], MDT, tag="k")
            for mat, dst, bcol in ((0, q_t, MT + FT + MT), (1, k_t, MT + FT + 2 * MT)):
                for mt in range(MT):
                    ps = pbig.tile([128, 512], F32, tag="pbig")
                    for kc in range(KC):
                        mm(ps[:, 0:PW], wqkv[:, mat, kc, mt * 128:(mt + 1) * 128],
                           h[:, kc, :], first=(kc == 0), last=(kc == KC - 1))
                    nc.vector.tensor_scalar(
                        dst[:, mt, :], ps[:, 0:PW],
                        biasc[:, bcol + mt:bcol + mt + 1], None, op0=OP.add)
            v_t = v_tiles[p % 2]
            for tb in range(2 * nb_in_p):
                vps = pbig.tile([128, 512], F32, tag="pbig")
                for kc in range(KC):
                    mm(vps[:, 0:D], h[:, kc, tb * 128:(tb + 1) * 128],
                       wqkv[:, 2, kc, :], first=(kc == 0), last=(kc == KC - 1))
                nc.vector.tensor_tensor(
                    v_t[:, tb, :, 0:HD],
                    vps[:, 0:D].rearrange("p (h d) -> p h d", h=n_heads),
                    vbias[:, :].rearrange("p (h d) -> p h d", h=n_heads),
                    op=OP.add)
            mids_out = run(mid_a)
            weave = fill_factory(mids_out) if fill_factory is not None else fill
            pad_mm = weave.pad_mm if weave is not None else (lambda: None)
            pad_relu = weave.pad_relu if weave is not None else (lambda: None)
            atms = []
            for bi in range(nb_in_p):
                boff = bi * T
                us = upool.tile([128, n_heads, 384], MDT, tag="u")
                pv0 = ppv.tile([128, n_heads, HD1], F32, tag="ppv")
                pv1 = ppv.tile([128, n_heads, HD1], F32, tag="ppv")

                def emit_S(hh):
                    # us[:, hh, 0:256]  = S_T[s in chunk0, t 0:256]
                    # us[:, hh, 256:384] = S_T[s in chunk1, t 128:256]
                    hp = 64 * (hh % 2)
                    hc = hh // 2
                    sps = pbig.tile([128, 384], F32, tag="pbig")
                    mm(sps[:, 0:256], k_t[hp:hp + HD, hc, boff:boff + 128],
                       q_t[hp:hp + HD, hc, boff:boff + T])
                    mm(sps[:, 256:384], k_t[hp:hp + HD, hc, boff + 128:boff + 256],
                       q_t[hp:hp + HD, hc, boff + 128:boff + 256])
                    nc.scalar.activation(us[:, hh, :], sps[:, :], AF.Exp)

                def emit_mask(h0):
                    # zero the masked upper-triangles of the two diagonal
                    # blocks (cols 0:128 and 256:384); cols 128:256 are the
                    # fully-live (s chunk0, t 128:256) block (mask==1 there)
                    nc.vector.tensor_tensor(
                        us[:, h0:h0 + 3, :], us[:, h0:h0 + 3, :],
                        mask[:, None, :].broadcast_to([128, 3, 384]), op=OP.mult)

                def emit_PV(hh):
                    mm(pv0[:, hh, :], us[:, hh, 0:128], v_t[:, 2 * bi, hh, :])
                    mm(pv1[:, hh, :], us[:, hh, 128:256], v_t[:, 2 * bi, hh, :],
                       first=True, last=False)
                    mm(pv1[:, hh, :], us[:, hh, 256:384], v_t[:, 2 * bi + 1, hh, :],
                       first=False, last=True)

                emit_S(0); emit_S(1); emit_S(2)
                pad_relu()   # flush previous bi's deferred relus (hidden by exps)
                pad_mm()
                emit_mask(0)
                emit_PV(0); emit_S(3)
                emit_PV(1); emit_S(4)
                emit_PV(2); emit_S(5)
                emit_mask(3)
                pad_mm()
                emit_PV(3); emit_PV(4); emit_PV(5)
                atm = atmpool.tile([128, 2, n_heads * HD], MDT, tag="atm")
                for tb, pv in ((0, pv0), (1, pv1)):
                    rden = rdpool.tile([128, n_heads], F32, tag="rd")
                    nc.vector.reciprocal(rden[:, :], pv[:, :, HD])
                    nc.vector.tensor_tensor(
                        atm[:, tb, :].rearrange("p (h d) -> p h d", h=n_heads),
                        pv[:, :, 0:HD],
                        rden[:, :, None].broadcast_to([128, n_heads, HD]),
                        op=OP.mult)
                atms.append(atm)
            att_t = attpool.tile([128, KC, PW], MDT, tag="att")
            for c in range(KC):
                tps = pbig.tile([128, 512], MDT, tag="pbig")
                for bi in range(nb_in_p):
                    for tb in range(2):
                        col = (bi * 2 + tb) * 128
                        nc.tensor.transpose(
                            tps[:, col:col + 128],
                            atms[bi][:, tb, c * 128:(c + 1) * 128],
                            ident[:, :])
                nc.vector.tensor_copy(att_t[:, c, :], tps[:, 0:PW])
            for mt in range(MT):
                pp = pbig.tile([128, 512], F32, tag="pbig")
                for kc in range(KC):
                    mm(pp[:, 0:PW], wproj[:, kc, mt * 128:(mt + 1) * 128],
                       att_t[:, kc, :], first=(kc == 0), last=(kc == KC - 1))
                nc.vector.scalar_tensor_tensor(
                    x[:, mt, pc], pp[:, 0:PW], biasc[:, mt:mt + 1], x[:, mt, pc],
                    op0=OP.add, op1=OP.add)
            if weave is not None:
                weave.drain()
            return mids_out

        # ---------------- phase D: FFN for a pair ----------------
        def emit_D(p, h2, w1, w2, biasc, mid=()):
            pc = slice(p * PW, p * PW + PW)
            h1_t = h1pool.tile([128, FT, PW], MDT, tag="h1")
            for mt in range(FT):
                fps = pbig.tile([128, 512], F32, tag="pbig")
                for kc in range(KC):
                    mm(fps[:, 0:PW], w1[:, kc, mt * 128:(mt + 1) * 128],
                       h2[:, kc, :], first=(kc == 0), last=(kc == KC - 1))
                nc.scalar.activation(h1_t[:, mt, :], fps[:, 0:PW], AF.Relu,
                                     bias=biasc[:, MT + mt:MT + mt + 1])
            mids_out = run(mid)
            for mt in range(MT):
                fp2 = pbig.tile([128, 512], F32, tag="pbig")
                for kc in range(FT):
                    mm(fp2[:, 0:PW], w2[:, kc, mt * 128:(mt + 1) * 128],
                       h1_t[:, kc, :], first=(kc == 0), last=(kc == FT - 1))
                nc.vector.scalar_tensor_tensor(
                    x[:, mt, pc], fp2[:, 0:PW],
                    biasc[:, MT + FT + mt:MT + FT + mt + 1],
                    x[:, mt, pc], op0=OP.add, op1=OP.add)
            return mids_out

        # ---------------- final LN + head (emitted per pair, interleaved) ----
        def head_apply(tok):
            p = tok[0]
            hf = ln_finish(tok)
            for bi in range(PW // T):
                b = p * (PW // T) + bi
                hps = ppv.tile([V, 256], F32, tag="ppv")
                for kc in range(KC):
                    mm(hps[:, :], whead[:, kc, :], hf[:, kc, bi * T:(bi + 1) * T],
                       first=(kc == 0), last=(kc == KC - 1))
                lg = lgpool.tile([V, T], F32, tag="lg")
                nc.vector.tensor_scalar(lg[:, :], hps[:, :], bhead[0:V, 0:1], None,
                                        op0=OP.add)
                nc.sync.dma_start(out=out_d[b], in_=lg[:, :])

        # ---------------- layers: software-pipelined emission ----------------
        def fetch_weights(l):
            wqkv = wpool_qkv.tile([128, 3, KC, D], MDT, tag="wqkv")
            nc.sync.dma_start(out=wqkv[:, :, :, :], in_=wqkv_d[l])
            wproj = wpool_proj.tile([128, KC, D], MDT, tag="wproj")
            nc.sync.dma_start(out=wproj[:, :, :], in_=wproj_d[l])
            w1 = wpool_1.tile([128, KC, DFF], MDT, tag="w1")
            nc.sync.dma_start(out=w1[:, :, :], in_=w1_d[l])
            w2 = wpool_2.tile([128, FT, D], MDT, tag="w2")
            nc.sync.dma_start(out=w2[:, :, :], in_=w2_d[l])
            vbias = wpool_b.tile([128, D], F32, tag="vbias")
            nc.sync.dma_start(out=vbias[:, :], in_=vbias_d[l])
            biasc = wpool_b.tile([128, NBC], F32, tag="biasc")
            nc.sync.dma_start(out=biasc[:, :], in_=biasc_d[l])
            return wqkv, wproj, w1, w2, vbias, biasc

        carry = {}
        wcur = fetch_weights(0)
        for l in range(n_layers):
            wqkv, wproj, w1, w2, vbias, biasc = wcur

            Bp = lambda p, h, **kw: emit_B(p, h, wqkv, wproj, vbias, biasc, **kw)
            Dp = lambda p, h2, **kw: emit_D(p, h2, w1, w2, biasc, **kw)
            mkD = lambda p, h2, w1o, w2o, bco, mid=(): DWeave(p, h2, w1o, w2o, bco, mid)

            if npair == 4:
                # steady state: B(p) weaves D(p-1)'s FFN chains into its
                # attention stalls; D(3) is split across the layer boundary.
                if l == 0:
                    h0 = ln_finish(ln_stats(0))
                    s1 = ln_stats(1)
                    d3w = None
                else:
                    h0, s1, d3w = carry["h0"], carry["s1"], carry["d3w"]
                (h1,) = Bp(0, h0, mid_a=[lambda: ln_finish(s1)], fill=d3w)
                if d3w is not None:
                    d3w.drain()
                sc0 = ln_stats(0)
                w0 = {}
                h2_0, s2 = Bp(1, h1,
                              mid_a=[lambda: ln_finish(sc0),
                                     lambda: ln_stats(2)],
                              fill_factory=lambda mids: w0.setdefault("w", mkD(
                                  0, mids[0], w1, w2, biasc,
                                  mid=[lambda: ln_finish(mids[1])])))
                (g2,) = w0["w"].mids
                sc1 = ln_stats(1)
                wn = {}
                w1_ = {}
                h2_1, s3, _w = Bp(2, g2,
                                  mid_a=[lambda: ln_finish(sc1),
                                         lambda: ln_stats(3),
                                         lambda: (l + 1 < n_layers)
                                         and fetch_weights(l + 1)],
                                  fill_factory=lambda mids: w1_.setdefault("w", mkD(
                                      1, mids[0], w1, w2, biasc,
                                      mid=[lambda: ln_finish(mids[1])])))
                wnext = _w
                (g3,) = w1_["w"].mids
                sc2 = ln_stats(2)
                w2_ = {}
                (h2_2,) = Bp(3, g3, mid_a=[lambda: ln_finish(sc2)],
                             fill_factory=lambda mids: w2_.setdefault("w", mkD(
                                 2, mids[0], w1, w2, biasc)))
                last = (l == n_layers - 1)
                if not last:
                    wcur = wnext
                    sc3 = ln_stats(3)
                    s0n = ln_stats(0)
                    h2_3 = ln_finish(sc3)
                    d3w = mkD(3, h2_3, w1, w2, biasc)
                    d3w.drain(n_f1=8, f2=False)   # feed the PE across the boundary
                    h0n = ln_finish(s0n)
                    carry = {"h0": h0n, "s1": ln_stats(1), "d3w": d3w}
                else:
                    # pair p's x is final after Dp(p): interleave the final
                    # LN + head work into the remaining FFN phases
                    sc3 = ln_stats(3)
                    sh0 = ln_stats(0)
                    sh1 = ln_stats(1)
                    h2_3 = ln_finish(sc3)
                    _, _, sh2 = Dp(3, h2_3, mid=[lambda: head_apply(sh0),
                                                 lambda: head_apply(sh1),
                                                 lambda: ln_stats(2)])
                    head_apply(sh2)
                    head_apply(ln_stats(3))
            else:
                # simple order for small test configs
                hq = {}
                hq[0] = ln_finish(ln_stats(0))
                for p in range(1, npair):
                    hq[p] = ln_finish(ln_stats(p))
                    Bp(p - 1, hq.pop(p - 1))
                Bp(npair - 1, hq.pop(npair - 1))
                hq[0] = ln_finish(ln_stats(0))
                for p in range(1, npair):
                    hq[p] = ln_finish(ln_stats(p))
                    Dp(p - 1, hq.pop(p - 1))
                Dp(npair - 1, hq.pop(npair - 1))

        # ---------------- final LN + head (small configs only) ----------------
        if npair != 4:
            for p in range(npair):
                head_apply(ln_stats(p))

    nc.compile()
    return nc


# ---------------------------------------------------------------------------
# host side
# ---------------------------------------------------------------------------

def prep_inputs(inputs, n_layers=L, n_b=BL, core=0):
    """Build the per-core input map (numpy) for `core`."""
    f32 = np.float32
    idx = np.asarray(inputs["idx"])
    tok_emb = np.asarray(inputs["tok_emb"], f32)
    pos_emb = np.asarray(inputs["pos_emb"], f32)
    Wq = np.asarray(inputs["Wq"], f32)
    Wk = np.asarray(inputs["Wk"], f32)
    Wv = np.asarray(inputs["Wv"], f32)
    Wproj = np.asarray(inputs["Wproj"], f32)
    bproj = np.asarray(inputs["bproj"], f32)
    W1 = np.asarray(inputs["W1"], f32)
    b1 = np.asarray(inputs["b1"], f32)
    W2 = np.asarray(inputs["W2"], f32)
    b2 = np.asarray(inputs["b2"], f32)
    ln1_g = np.asarray(inputs["ln1_g"], f32)
    ln1_b = np.asarray(inputs["ln1_b"], f32)
    ln2_g = np.asarray(inputs["ln2_g"], f32)
    ln2_b = np.asarray(inputs["ln2_b"], f32)
    lnf_g = np.asarray(inputs["lnf_g"], f32)
    lnf_b = np.asarray(inputs["lnf_b"], f32)
    Whead = np.asarray(inputs["Whead"], f32)
    bhead = np.asarray(inputs["bhead"], f32)

    ntok = n_b * T
    scale = f32(D) ** -0.5

    idx_c = idx[core * n_b:(core + 1) * n_b].reshape(-1)         # [ntok]
    hot = (idx_c[None, :] == np.arange(V)[:, None]).astype(f32)  # [V, ntok]

    # center embeddings per token/position so x0 is feature-mean-free
    tok_emb = tok_emb - tok_emb.mean(axis=1, keepdims=True)
    pos_emb = pos_emb - pos_emb.mean(axis=1, keepdims=True)

    posT = pos_emb.T.astype(f32)                                 # [D, T]
    posT2 = np.concatenate([posT, posT], axis=1)                 # [D, 512]
    pos_in = posT2.reshape(KC, 128, 512).transpose(1, 0, 2).copy()

    lane = np.arange(128)
    t = np.arange(T)
    tri0 = (lane[:, None] <= t[None, :128]).astype(f32)          # diag block
    mask = np.concatenate([tri0, np.ones((128, 128), f32), tri0], axis=1)

    def pack_w(w):  # [D_in, N] -> [128, KC_in, N]
        kin = w.shape[0] // 128
        return w.reshape(kin, 128, -1).transpose(1, 0, 2).copy()

    wqkv = np.zeros((n_layers, 128, 3, KC, D), f32)
    wproj = np.zeros((n_layers, 128, KC, D), f32)
    w1 = np.zeros((n_layers, 128, KC, DFF), f32)
    w2 = np.zeros((n_layers, 128, FT, D), f32)
    vbias = np.zeros((n_layers, 128, D), f32)
    biasc = np.zeros((n_layers, 128, NBC), f32)

    # The feature-mean of the residual stream is kept identically zero by
    # column-centering every contribution to it (embeddings + proj/ffn2
    # outputs): LN is invariant to per-token mean shifts of its input, so
    # results are unchanged, and the kernel's LN needs no mean statistics.
    # Additionally every weight consuming a post-LN activation is centered
    # along its input dim: (x - mean(x)) @ Wg == x @ (Wg - colmean(Wg)).
    center_in = lambda w: w - w.mean(axis=0, keepdims=True)    # input dim
    center_out = lambda w: w - w.mean(axis=1, keepdims=True)   # output dim

    for l in range(n_layers):
        # Wq[l] is [H, D, HD]; feature f = h*HD+hd -> transpose to [D, H, HD]
        wq2 = Wq[l].transpose(1, 0, 2).reshape(D, D) * scale
        wk2 = Wk[l].transpose(1, 0, 2).reshape(D, D)
        wv2 = Wv[l].transpose(1, 0, 2).reshape(D, D)
        wqkv[l, :, 0] = pack_w(center_in(wq2 * ln1_g[l][:, None]))
        wqkv[l, :, 1] = pack_w(center_in(wk2 * ln1_g[l][:, None]))
        wqkv[l, :, 2] = pack_w(center_in(wv2 * ln1_g[l][:, None]))
        vbias[l] = np.broadcast_to(ln1_b[l] @ wv2, (128, D))
        wproj[l] = pack_w(center_out(Wproj[l]))
        w1[l] = pack_w(center_in(W1[l] * ln2_g[l][:, None]))
        w2[l] = pack_w(center_out(W2[l]))
        biasc[l, :, 0:MT] = (bproj[l] - bproj[l].mean()).reshape(MT, 128).T
        biasc[l, :, MT:MT + FT] = (b1[l] + ln2_b[l] @ W1[l]).reshape(FT, 128).T
        biasc[l, :, MT + FT:MT + FT + MT] = \
            (b2[l] - b2[l].mean()).reshape(MT, 128).T
        biasc[l, :, MT + FT + MT:MT + FT + 2 * MT] = \
            (ln1_b[l] @ wq2).reshape(MT, 128).T
        biasc[l, :, MT + FT + 2 * MT:] = (ln1_b[l] @ wk2).reshape(MT, 128).T

    whead_eff = center_in(Whead * lnf_g[:, None])
    bhead_eff = (bhead + lnf_b @ Whead).astype(f32)

    cst = np.ones((128, CST_W), f32)
    cst[:, 512] = 1.0 / D
    cst[:, 513:641] = np.eye(128, dtype=f32)
    cst[:, 641:769] = 1.0 / 512  # exact in bf16; 512/D folded into Ln scale

    bf = lambda a: np.ascontiguousarray(a).astype(NPBF)

    return {
        "cst": bf(cst),
        "epsc": np.full((128, 1), EPS, f32),
        "hotT": bf(hot),
        "temb": bf(tok_emb),
        "posT": bf(pos_in),
        "mask": bf(mask),
        "wqkv": bf(wqkv),
        "wproj": bf(wproj),
        "w1": bf(w1),
        "w2": bf(w2),
        "vbias": vbias,
        "biasc": biasc,
        "whead": bf(pack_w(whead_eff)),
        "bheadc": bhead_eff[:, None].copy(),
    }


_CACHE = {}


def get_program():
    if "nc" not in _CACHE:
        _CACHE["nc"] = build_program()
    return _CACHE["nc"]


def run_on_hw(inputs, trace=False):
    nc = get_program()
    in_maps = [prep_inputs(inputs, core=c) for c in range(NCORES)]
    res = run_bass_kernel_spmd(nc, in_maps, list(range(NCORES)), trace=trace)
    outs = []
    for c in range(NCORES):
        lt = res.results[c]["logitsT"]          # [BL, V, T]
        outs.append(lt.transpose(0, 2, 1))      # [BL, T, V]
    full = np.concatenate(outs, axis=0)         # [B, T, V]
    return full, res


def kernel(**inputs):
    out, _ = run_on_hw(inputs, trace=False)
    return out


# revision 42
# speedup vs baseline: 1.5009x; 1.0237x over previous
"""Trainium2 Bass kernel: GPT-style transformer forward pass.

Strategy: data-parallel over batch across 8 NeuronCores (B=64 -> 8 per core),
weights replicated.  On each core, activations are kept feature-major
(x_T [D=384 (3x128 partitions), 2048 tokens]) so matmuls contract the
partition dim with no activation transposes; LN/QKV/proj/FFN run on
512-token pairs of batch elements.  All matmul operands are bfloat16 (PSUM
accumulation stays fp32): on TRN2 hardware bf16 streams the PE at 2.4 GHz
while fp32r runs at the 1.2 GHz "others" clock, so bf16 doubles matmul
throughput on top of halving LDWEIGHTS and weight DMA.  The fp32 residual
stream x is kept in SBUF; a bf16 shadow is cast on the (otherwise idle)
GPSIMD engine for the LN mean matmuls.  LayerNorm stats come from
ones-vector matmuls (partition reduction); rstd = exp(-0.5*ln(var+eps)) on
the Scalar engine so the whole kernel lives in the natural_log_exp
activation table set.  Attention computes transposed scores S_T[s,t] per
(b,h) for the three live causal blocks only, exp on Scalar, one grouped
multiplicative mask per 3 heads on Vector, then token-major PV with an
appended ones-column in V so softmax denominators land as a per-partition
column, and PE transposes carry att back to feature-major for the
projection.  Q/K biases ride the PSUM->SBUF copy as tensor_scalar adds
instead of rank-1 matmuls.
"""

import os
import sys

for _p in ("/opt/trn_rl_repo",):
    if _p not in sys.path and os.path.isdir(_p):
        sys.path.insert(0, _p)

import numpy as np
import ml_dtypes

import concourse.bass as bass
import concourse.mybir as mybir
import concourse.tile as tile
from concourse import bacc
from concourse.bass_utils import run_bass_kernel_spmd

F32 = mybir.dt.float32
F32R = mybir.dt.float32r
BF16 = mybir.dt.bfloat16
NPBF = ml_dtypes.bfloat16
AF = mybir.ActivationFunctionType
OP = mybir.AluOpType

V, D, H, HD, L, T, B = 65, 384, 6, 64, 6, 256, 64
NCORES = 8
BL = B // NCORES          # batch elements per core
NTOK = BL * T             # tokens per core
DFF = 4 * D               # 1536
EPS = 1e-3
KC = D // 128             # 3 contraction chunks of 128
MT = D // 128             # 3 output feature tiles
FT = DFF // 128           # 12 ffn tiles
HD1 = HD + 2              # V columns incl ones col (64) + pad (65)
NBC = MT + FT + MT + MT + MT  # biasc cols: proj, ffn1, ffn2, q, k

MDT = BF16

# cst layout: cols 0:512 ones, col 512 = 1/D, cols 513:641 identity 128x128,
# cols 641:769 = 1/D block (stats matmul lhsT)
CST_W = 513 + 128 + 128


def _patch_act_tables():
    """Steer the activation-table picker to natural_log_exp_and_others for
    Exp and Ln, so this kernel's ACT stream never switches table sets."""
    if getattr(bacc, "_act_tables_patched", False):
        return
    real = bacc.get_activation_tables

    def patched(arch):
        t = real(arch)
        exp = mybir.ActivationFunctionType.Exp
        ln = mybir.ActivationFunctionType.Ln
        out = {}
        for name, fns in t.items():
            if name != "natural_log_exp_and_others":
                fns = fns - {exp, ln}
            out[name] = fns
        return out

    bacc.get_activation_tables = patched
    bacc._act_tables_patched = True


class _MM:
    """matmul emitter with explicit accumulation-chain boundaries."""

    def __init__(self, nc):
        self.nc = nc

    def __call__(self, out, lhsT, rhs, first=True, last=True, tile_position=None):
        self.nc.tensor.matmul(
            out, lhsT, rhs, start=first, stop=last, tile_position=tile_position,
        )


def build_program(n_layers=L, n_b=BL, n_heads=H):
    _patch_act_tables()
    assert n_b % 2 == 0 or n_b == 1
    ntok = n_b * T
    npair = max(1, n_b // 2)
    PW = 512 if n_b > 1 else 256      # tokens per pair-chunk
    nc = bacc.Bacc("TRN2", target_bir_lowering=False, debug=False)

    # ---------------- DRAM I/O ----------------
    hot_d = nc.dram_tensor("hotT", [V, ntok], MDT, kind="ExternalInput").ap()
    temb_d = nc.dram_tensor("temb", [V, D], MDT, kind="ExternalInput").ap()
    pos_d = nc.dram_tensor("posT", [128, KC, 512], MDT, kind="ExternalInput").ap()
    mask_d = nc.dram_tensor("mask", [128, 384], MDT, kind="ExternalInput").ap()
    wqkv_d = nc.dram_tensor("wqkv", [n_layers, 128, 3, KC, D], MDT, kind="ExternalInput").ap()
    wproj_d = nc.dram_tensor("wproj", [n_layers, 128, KC, D], MDT, kind="ExternalInput").ap()
    w1_d = nc.dram_tensor("w1", [n_layers, 128, KC, DFF], MDT, kind="ExternalInput").ap()
    w2_d = nc.dram_tensor("w2", [n_layers, 128, FT, D], MDT, kind="ExternalInput").ap()
    vbias_d = nc.dram_tensor("vbias", [n_layers, 128, D], F32, kind="ExternalInput").ap()
    biasc_d = nc.dram_tensor("biasc", [n_layers, 128, NBC], F32, kind="ExternalInput").ap()
    whead_d = nc.dram_tensor("whead", [128, KC, V], MDT, kind="ExternalInput").ap()
    bhead_d = nc.dram_tensor("bheadc", [V, 1], F32, kind="ExternalInput").ap()
    cst_d = nc.dram_tensor("cst", [128, CST_W], MDT, kind="ExternalInput").ap()
    epsc_d = nc.dram_tensor("epsc", [128, 1], F32, kind="ExternalInput").ap()
    out_d = nc.dram_tensor("logitsT", [n_b, V, T], F32, kind="ExternalOutput").ap()

    from contextlib import ExitStack

    with tile.TileContext(nc) as tc, \
         nc.allow_low_precision(reason="bf16 matmul operand production"), \
         ExitStack() as ctx:
        ep = ctx.enter_context

        # ---------------- pools ----------------
        cpool = ep(tc.tile_pool(name="consts", bufs=1))
        xpool = ep(tc.tile_pool(name="x", bufs=1))
        wpool_qkv = ep(tc.tile_pool(name="wqkv", bufs=2))
        wpool_proj = ep(tc.tile_pool(name="wproj", bufs=2))
        wpool_1 = ep(tc.tile_pool(name="w1", bufs=2))
        wpool_2 = ep(tc.tile_pool(name="w2", bufs=2))
        wpool_b = ep(tc.tile_pool(name="wbias", bufs=2))
        hpool = ep(tc.tile_pool(name="h", bufs=3))
        xsqpool = ep(tc.tile_pool(name="xsq", bufs=2))
        qpool = ep(tc.tile_pool(name="q", bufs=2))
        kpool = ep(tc.tile_pool(name="k", bufs=2))
        vpool = ep(tc.tile_pool(name="v", bufs=1))
        upool = ep(tc.tile_pool(name="u", bufs=2))
        atmpool = ep(tc.tile_pool(name="atm", bufs=2))
        attpool = ep(tc.tile_pool(name="att", bufs=1))
        h1pool = ep(tc.tile_pool(name="h1", bufs=2))
        lgpool = ep(tc.tile_pool(name="lg", bufs=1))
        stpool = ep(tc.tile_pool(name="st", bufs=6))
        rdpool = ep(tc.tile_pool(name="rd", bufs=4))

        pbig = ep(tc.tile_pool(name="pbig", bufs=5, space="PSUM"))
        ppv = ep(tc.tile_pool(name="ppv", bufs=2, space="PSUM"))
        pstat = ep(tc.tile_pool(name="pstat", bufs=1, space="PSUM"))

        mm = _MM(nc)

        # ---------------- constants (embedding-critical DMAs first) ----------------
        temb = stpool.tile([V, 384], MDT, tag="st")
        nc.sync.dma_start(out=temb[:, :], in_=temb_d[:, :])
        hots = []
        for ch in range(ntok // PW):
            hot = xsqpool.tile([V, PW], MDT, tag="xsq")
            nc.sync.dma_start(out=hot[:, :], in_=hot_d[:, ch * PW:(ch + 1) * PW])
            hots.append(hot)
        pos = attpool.tile([128, KC, 512], MDT, tag="att")
        nc.sync.dma_start(out=pos[:, :, :], in_=pos_d[:, :, :])
        cst = cpool.tile([128, CST_W], MDT, name="cst_c")
        nc.sync.dma_start(out=cst[:, :], in_=cst_d[:, :])
        ones = cst[:, 0:512]
        ident = cst[:, 513:641]
        invD = cst[:, 641:769]
        epsc = cpool.tile([128, 1], F32, name="epsc_c")
        nc.sync.dma_start(out=epsc[:, :], in_=epsc_d[:, :])
        mask = cpool.tile([128, 384], MDT, name="mask_c")
        nc.sync.dma_start(out=mask[:, :], in_=mask_d[:, :])
        whead = cpool.tile([128, KC, V], MDT, name="whead_c")
        nc.sync.dma_start(out=whead[:, :, :], in_=whead_d[:, :, :])
        bhead = cpool.tile([V, 1], F32, name="bhead_c")
        nc.sync.dma_start(out=bhead[:, :], in_=bhead_d[:, :])

        x = xpool.tile([128, KC, ntok], F32R, name="x_resid")

        nbp = PW // T
        v_tiles = [vpool.tile([128, 2 * nbp, n_heads, HD1], MDT, name=f"v_pp{i}")
                   for i in range(2)]
        for vt in v_tiles:
            for tb in range(2 * nbp):
                nc.vector.tensor_copy(vt[:, tb, :, HD:HD1],
                                      ones[:, 0:2 * n_heads].rearrange(
                                          "p (h c) -> p h c", h=n_heads))

        # ---------------- embedding ----------------
        for ch in range(ntok // PW):
            cs = slice(ch * PW, ch * PW + PW)
            hot = hots[ch]
            for c in range(KC):
                ps = pbig.tile([128, 512], F32, tag="pbig")
                mm(ps[:, 0:PW], temb[0:V, c * 128:(c + 1) * 128], hot[0:V, :])
                nc.vector.tensor_tensor(x[:, c, cs], ps[:, 0:PW], pos[:, c, 0:PW], op=OP.add)

        # ---------------- LN split into stats + finish ----------------
        def ln_stats(p):
            """square (ACT) + meansq matmuls (arrive pre-broadcast across
            partitions) + Ln/Exp rstd chain.  Every residual-stream
            contribution is column-centered host-side, so the feature-mean
            of x is identically zero: no mean chain, var = E[x^2]."""
            pc = slice(p * PW, p * PW + PW)
            xsq = xsqpool.tile([128, KC, PW], MDT, tag="xsq")
            nc.scalar.activation(xsq[:, :, :], x[:, :, pc], AF.Square)
            msqb = pstat.tile([128, PW], F32, tag="pstat")
            for c in range(KC):
                mm(msqb[:, :], invD[:, :], xsq[:, c, :],
                   first=(c == 0), last=(c == KC - 1))
            # invD is 1/512 (exact in bf16); rescale by 512/D inside the Ln
            varb = stpool.tile([128, PW], F32, tag="st")
            nc.scalar.activation(varb[:, :], msqb[:, :], AF.Ln,
                                 bias=epsc[:, 0:1], scale=512.0 / D)
            rstdb = stpool.tile([128, PW], F32, tag="st")
            nc.scalar.activation(rstdb[:, :], varb[:, :], AF.Exp, scale=-0.5)
            return (p, rstdb)

        def ln_finish(tok):
            """apply -> h (one DVE op; mean handled by centered weights)"""
            p, rstdb = tok
            pc = slice(p * PW, p * PW + PW)
            h = hpool.tile([128, KC, PW], MDT, tag="h")
            nc.vector.tensor_tensor(
                h[:, :, :], x[:, :, pc],
                rstdb[:, None, :].broadcast_to([128, KC, PW]), op=OP.mult)
            return h

        def run(mids):
            out = []
            for f in mids:
                out.append(f())
            return out

        # ---------------- FFN as a weavable unit stream ----------------
        class DWeave:
            """FFN work for one pair, consumable in-order as filler units.
            pad_mm() emits one ffn1 3-matmul chain (pure PE) and defers its
            relu; pad_relu() flushes deferred relus onto DVE (so the ACT exp
            stream in the attention phase is never delayed); drain() finishes
            everything (remaining relus on ACT)."""

            def __init__(self, p, h2, w1, w2, biasc, mid=(), hold_f2=False):
                self.p = p
                self.pc = slice(p * PW, p * PW + PW)
                self.h2 = h2
                self.w1, self.w2, self.biasc = w1, w2, biasc
                self.mid = mid
                self.hold_f2 = hold_f2
                self.mids = []
                self.h1_t = h1pool.tile([128, FT, PW], MDT, tag="h1")
                self.fps = {}
                self.i1 = 0          # next ffn1 chain to emit
                self.pending = []    # ffn1 tiles awaiting relu
                self.mid_done = False
                self.i2 = 0          # next ffn2 chain to emit

            def _chain1(self, mt):
                fps = pbig.tile([128, 512], F32, tag="pbig")
                for kc in range(KC):
                    mm(fps[:, 0:PW], self.w1[:, kc, mt * 128:(mt + 1) * 128],
                       self.h2[:, kc, :], first=(kc == 0), last=(kc == KC - 1))
                return fps

            def _relu(self, mt, fps, dve):
                bcol = self.biasc[:, MT + mt:MT + mt + 1]
                if dve:
                    nc.vector.tensor_scalar(
                        self.h1_t[:, mt, :], fps[:, 0:PW], bcol, 0.0,
                        op0=OP.add, op1=OP.max)
                else:
                    nc.scalar.activation(self.h1_t[:, mt, :], fps[:, 0:PW],
                                         AF.Relu, bias=bcol)

            def pad_mm(self):
                if self.i1 < FT and len(self.pending) < 2:
                    mt = self.i1
                    self.i1 += 1
                    self.pending.append((mt, self._chain1(mt)))

            def pad_relu(self):
                for mt, fps in self.pending:
                    self._relu(mt, fps, dve=True)
                self.pending = []

            def _chain2(self, mt):
                fp2 = pbig.tile([128, 512], F32, tag="pbig")
                for kc in range(FT):
                    mm(fp2[:, 0:PW], self.w2[:, kc, mt * 128:(mt + 1) * 128],
                       self.h1_t[:, kc, :], first=(kc == 0), last=(kc == FT - 1))
                nc.vector.scalar_tensor_tensor(
                    x[:, mt, self.pc], fp2[:, 0:PW],
                    self.biasc[:, MT + FT + mt:MT + FT + mt + 1],
                    x[:, mt, self.pc], op0=OP.add, op1=OP.add)

            def drain(self, n_f1=FT, f2=True):
                for mt, fps in self.pending:
                    self._relu(mt, fps, dve=True)
                self.pending = []
                while self.i1 < n_f1:
                    mt = self.i1
                    self.i1 += 1
                    self._relu(mt, self._chain1(mt), dve=False)
                if self.i1 < FT:
                    return
                if not self.mid_done:
                    self.mid_done = True
                    self.mids = run(self.mid)
                if f2:
                    while self.i2 < MT:
                        self._chain2(self.i2)
                        self.i2 += 1

        # ---------------- phase B: qkv + attention + proj for a pair ----------------
        def emit_B(p, h, wqkv, wproj, vbias, biasc, mid_a=(), fill=None,
                   fill_factory=None):
            pc = slice(p * PW, p * PW + PW)
            nb_in_p = PW // T
            q_t = qpool.tile([128, MT, PW], MDT, tag="q")
            k_t = kpool.tile([128, MT, PW], MDT, tag="k")
            for mat, dst, bcol in ((0, q_t, MT + FT + MT), (1, k_t, MT + FT + 2 * MT)):
                for mt in range(MT):
                    ps = pbig.tile([128, 512], F32, tag="pbig")
                    for kc in range(KC):
                        mm(ps[:, 0:PW], wqkv[:, mat, kc, mt * 128:(mt + 1) * 128],
                           h[:, kc, :], first=(kc == 0), last=(kc == KC - 1))
                    nc.vector.tensor_scalar(
                        dst[:, mt, :], ps[:, 0:PW],
                        biasc[:, bcol + mt:bcol + mt + 1], None, op0=OP.add)
            v_t = v_tiles[p % 2]
            for tb in range(2 * nb_in_p):
                vps = pbig.tile([128, 512], F32, tag="pbig")
                for kc in range(KC):
                    mm(vps[:, 0:D], h[:, kc, tb * 128:(tb + 1) * 128],
                       wqkv[:, 2, kc, :], first=(kc == 0), last=(kc == KC - 1))
                nc.vector.tensor_tensor(
                    v_t[:, tb, :, 0:HD],
                    vps[:, 0:D].rearrange("p (h d) -> p h d", h=n_heads),
                    vbias[:, :].rearrange("p (h d) -> p h d", h=n_heads),
                    op=OP.add)
            mids_out = run(mid_a)
            weave = fill_factory(mids_out) if fill_factory is not None else fill
            pad_mm = weave.pad_mm if weave is not None else (lambda: None)
            pad_relu = weave.pad_relu if weave is not None else (lambda: None)
            atms = []
            for bi in range(nb_in_p):
                boff = bi * T
                us = upool.tile([128, n_heads, 384], MDT, tag="u")
                pv0 = ppv.tile([128, n_heads, HD1], F32, tag="ppv")
                pv1 = ppv.tile([128, n_heads, HD1], F32, tag="ppv")

                def emit_S(hh):
                    # us[:, hh, 0:256]  = S_T[s in chunk0, t 0:256]
                    # us[:, hh, 256:384] = S_T[s in chunk1, t 128:256]
                    hp = 64 * (hh % 2)
                    hc = hh // 2
                    sps = pbig.tile([128, 384], F32, tag="pbig")
                    mm(sps[:, 0:256], k_t[hp:hp + HD, hc, boff:boff + 128],
                       q_t[hp:hp + HD, hc, boff:boff + T])
                    mm(sps[:, 256:384], k_t[hp:hp + HD, hc, boff + 128:boff + 256],
                       q_t[hp:hp + HD, hc, boff + 128:boff + 256])
                    nc.scalar.activation(us[:, hh, :], sps[:, :], AF.Exp)

                def emit_mask(h0):
                    # zero the masked upper-triangles of the two diagonal
                    # blocks (cols 0:128 and 256:384); cols 128:256 are the
                    # fully-live (s chunk0, t 128:256) block (mask==1 there)
                    nc.vector.tensor_tensor(
                        us[:, h0:h0 + 3, :], us[:, h0:h0 + 3, :],
                        mask[:, None, :].broadcast_to([128, 3, 384]), op=OP.mult)

                def emit_PV(hh):
                    mm(pv0[:, hh, :], us[:, hh, 0:128], v_t[:, 2 * bi, hh, :])
                    mm(pv1[:, hh, :], us[:, hh, 128:256], v_t[:, 2 * bi, hh, :],
                       first=True, last=False)
                    mm(pv1[:, hh, :], us[:, hh, 256:384], v_t[:, 2 * bi + 1, hh, :],
                       first=False, last=True)

                emit_S(0); emit_S(1); emit_S(2)
                pad_relu()   # flush previous bi's deferred relus (hidden by exps)
                pad_mm()
                emit_mask(0)
                emit_PV(0); emit_S(3)
                emit_PV(1); emit_S(4)
                emit_PV(2); emit_S(5)
                emit_mask(3)
                pad_mm()
                emit_PV(3); emit_PV(4); emit_PV(5)
                atm = atmpool.tile([128, 2, n_heads * HD], MDT, tag="atm")
                for tb, pv in ((0, pv0), (1, pv1)):
                    rden = rdpool.tile([128, n_heads], F32, tag="rd")
                    nc.vector.reciprocal(rden[:, :], pv[:, :, HD])
                    nc.vector.tensor_tensor(
                        atm[:, tb, :].rearrange("p (h d) -> p h d", h=n_heads),
                        pv[:, :, 0:HD],
                        rden[:, :, None].broadcast_to([128, n_heads, HD]),
                        op=OP.mult)
                atms.append(atm)
            att_t = attpool.tile([128, KC, PW], MDT, tag="att")
            for c in range(KC):
                tps = pbig.tile([128, 512], MDT, tag="pbig")
                for bi in range(nb_in_p):
                    for tb in range(2):
                        col = (bi * 2 + tb) * 128
                        nc.tensor.transpose(
                            tps[:, col:col + 128],
                            atms[bi][:, tb, c * 128:(c + 1) * 128],
                            ident[:, :])
                nc.vector.tensor_copy(att_t[:, c, :], tps[:, 0:PW])
            for mt in range(MT):
                pp = pbig.tile([128, 512], F32, tag="pbig")
                for kc in range(KC):
                    mm(pp[:, 0:PW], wproj[:, kc, mt * 128:(mt + 1) * 128],
                       att_t[:, kc, :], first=(kc == 0), last=(kc == KC - 1))
                nc.vector.scalar_tensor_tensor(
                    x[:, mt, pc], pp[:, 0:PW], biasc[:, mt:mt + 1], x[:, mt, pc],
                    op0=OP.add, op1=OP.add)
            if weave is not None:
                weave.drain(f2=not weave.hold_f2)
            return mids_out

        # ---------------- phase D: FFN for a pair ----------------
        def emit_D(p, h2, w1, w2, biasc, mid=()):
            pc = slice(p * PW, p * PW + PW)
            h1_t = h1pool.tile([128, FT, PW], MDT, tag="h1")
            for mt in range(FT):
                fps = pbig.tile([128, 512], F32, tag="pbig")
                for kc in range(KC):
                    mm(fps[:, 0:PW], w1[:, kc, mt * 128:(mt + 1) * 128],
                       h2[:, kc, :], first=(kc == 0), last=(kc == KC - 1))
                nc.scalar.activation(h1_t[:, mt, :], fps[:, 0:PW], AF.Relu,
                                     bias=biasc[:, MT + mt:MT + mt + 1])
            mids_out = run(mid)
            for mt in range(MT):
                fp2 = pbig.tile([128, 512], F32, tag="pbig")
                for kc in range(FT):
                    mm(fp2[:, 0:PW], w2[:, kc, mt * 128:(mt + 1) * 128],
                       h1_t[:, kc, :], first=(kc == 0), last=(kc == FT - 1))
                nc.vector.scalar_tensor_tensor(
                    x[:, mt, pc], fp2[:, 0:PW],
                    biasc[:, MT + FT + mt:MT + FT + mt + 1],
                    x[:, mt, pc], op0=OP.add, op1=OP.add)
            return mids_out

        # ---------------- final LN + head (emitted per pair, interleaved) ----
        def head_apply(tok):
            p = tok[0]
            hf = ln_finish(tok)
            for bi in range(PW // T):
                b = p * (PW // T) + bi
                hps = ppv.tile([V, 256], F32, tag="ppv")
                for kc in range(KC):
                    mm(hps[:, :], whead[:, kc, :], hf[:, kc, bi * T:(bi + 1) * T],
                       first=(kc == 0), last=(kc == KC - 1))
                lg = lgpool.tile([V, T], F32, tag="lg")
                nc.vector.tensor_scalar(lg[:, :], hps[:, :], bhead[0:V, 0:1], None,
                                        op0=OP.add)
                nc.sync.dma_start(out=out_d[b], in_=lg[:, :])

        # ---------------- layers: software-pipelined emission ----------------
        def fetch_weights(l):
            wqkv = wpool_qkv.tile([128, 3, KC, D], MDT, tag="wqkv")
            nc.sync.dma_start(out=wqkv[:, :, :, :], in_=wqkv_d[l])
            wproj = wpool_proj.tile([128, KC, D], MDT, tag="wproj")
            nc.sync.dma_start(out=wproj[:, :, :], in_=wproj_d[l])
            w1 = wpool_1.tile([128, KC, DFF], MDT, tag="w1")
            nc.sync.dma_start(out=w1[:, :, :], in_=w1_d[l])
            w2 = wpool_2.tile([128, FT, D], MDT, tag="w2")
            nc.sync.dma_start(out=w2[:, :, :], in_=w2_d[l])
            vbias = wpool_b.tile([128, D], F32, tag="vbias")
            nc.sync.dma_start(out=vbias[:, :], in_=vbias_d[l])
            biasc = wpool_b.tile([128, NBC], F32, tag="biasc")
            nc.sync.dma_start(out=biasc[:, :], in_=biasc_d[l])
            return wqkv, wproj, w1, w2, vbias, biasc

        carry = {}
        wcur = fetch_weights(0)
        for l in range(n_layers):
            wqkv, wproj, w1, w2, vbias, biasc = wcur

            Bp = lambda p, h, **kw: emit_B(p, h, wqkv, wproj, vbias, biasc, **kw)
            Dp = lambda p, h2, **kw: emit_D(p, h2, w1, w2, biasc, **kw)
            mkD = lambda p, h2, w1o, w2o, bco, **kw: DWeave(p, h2, w1o, w2o, bco, **kw)

            if npair == 4:
                # steady state: B(p) weaves D(p-1)'s FFN chains into its
                # attention stalls; D(3) is split across the layer boundary.
                if l == 0:
                    h0 = ln_finish(ln_stats(0))
                    s1 = ln_stats(1)
                    d3w = None
                else:
                    h0, s1, d3w = carry["h0"], carry["s1"], carry["d3w"]
                (h1,) = Bp(0, h0, mid_a=[lambda: ln_finish(s1)], fill=d3w)
                if d3w is not None:
                    d3w.drain()
                sc0 = ln_stats(0)
                w0 = {}
                h2_0, s2 = Bp(1, h1,
                              mid_a=[lambda: ln_finish(sc0),
                                     lambda: ln_stats(2)],
                              fill_factory=lambda mids: w0.setdefault("w", mkD(
                                  0, mids[0], w1, w2, biasc,
                                  mid=[lambda: ln_finish(mids[1])])))
                (g2,) = w0["w"].mids
                sc1 = ln_stats(1)
                wn = {}
                w1_ = {}
                h2_1, s3, _w = Bp(2, g2,
                                  mid_a=[lambda: ln_finish(sc1),
                                         lambda: ln_stats(3),
                                         lambda: (l + 1 < n_layers)
                                         and fetch_weights(l + 1)],
                                  fill_factory=lambda mids: w1_.setdefault("w", mkD(
                                      1, mids[0], w1, w2, biasc,
                                      mid=[lambda: ln_finish(mids[1])])))
                wnext = _w
                (g3,) = w1_["w"].mids
                sc2 = ln_stats(2)
                w2_ = {}
                (h2_2,) = Bp(3, g3, mid_a=[lambda: ln_finish(sc2)],
                             fill_factory=lambda mids: w2_.setdefault("w", mkD(
                                 2, mids[0], w1, w2, biasc, hold_f2=True)))
                last = (l == n_layers - 1)
                if not last:
                    wcur = wnext
                    sc3 = ln_stats(3)
                    s0n = ln_stats(0)
                    w2_["w"].drain()   # D(2) ffn2: PE work while LN chains run
                    h2_3 = ln_finish(sc3)
                    d3w = mkD(3, h2_3, w1, w2, biasc)
                    d3w.drain(n_f1=8, f2=False)   # feed the PE across the boundary
                    h0n = ln_finish(s0n)
                    carry = {"h0": h0n, "s1": ln_stats(1), "d3w": d3w}
                else:
                    # pair p's x is final after Dp(p): interleave the final
                    # LN + head work into the remaining FFN phases
                    sc3 = ln_stats(3)
                    sh0 = ln_stats(0)
                    sh1 = ln_stats(1)
                    w2_["w"].drain()
                    h2_3 = ln_finish(sc3)
                    _, _, sh2 = Dp(3, h2_3, mid=[lambda: head_apply(sh0),
                                                 lambda: head_apply(sh1),
                                                 lambda: ln_stats(2)])
                    head_apply(sh2)
                    head_apply(ln_stats(3))
            else:
                # simple order for small test configs
                hq = {}
                hq[0] = ln_finish(ln_stats(0))
                for p in range(1, npair):
                    hq[p] = ln_finish(ln_stats(p))
                    Bp(p - 1, hq.pop(p - 1))
                Bp(npair - 1, hq.pop(npair - 1))
                hq[0] = ln_finish(ln_stats(0))
                for p in range(1, npair):
                    hq[p] = ln_finish(ln_stats(p))
                    Dp(p - 1, hq.pop(p - 1))
                Dp(npair - 1, hq.pop(npair - 1))

        # ---------------- final LN + head (small configs only) ----------------
        if npair != 4:
            for p in range(npair):
                head_apply(ln_stats(p))

    nc.compile()
    return nc


# ---------------------------------------------------------------------------
# host side
# ---------------------------------------------------------------------------

def prep_inputs(inputs, n_layers=L, n_b=BL, core=0):
    """Build the per-core input map (numpy) for `core`."""
    f32 = np.float32
    idx = np.asarray(inputs["idx"])
    tok_emb = np.asarray(inputs["tok_emb"], f32)
    pos_emb = np.asarray(inputs["pos_emb"], f32)
    Wq = np.asarray(inputs["Wq"], f32)
    Wk = np.asarray(inputs["Wk"], f32)
    Wv = np.asarray(inputs["Wv"], f32)
    Wproj = np.asarray(inputs["Wproj"], f32)
    bproj = np.asarray(inputs["bproj"], f32)
    W1 = np.asarray(inputs["W1"], f32)
    b1 = np.asarray(inputs["b1"], f32)
    W2 = np.asarray(inputs["W2"], f32)
    b2 = np.asarray(inputs["b2"], f32)
    ln1_g = np.asarray(inputs["ln1_g"], f32)
    ln1_b = np.asarray(inputs["ln1_b"], f32)
    ln2_g = np.asarray(inputs["ln2_g"], f32)
    ln2_b = np.asarray(inputs["ln2_b"], f32)
    lnf_g = np.asarray(inputs["lnf_g"], f32)
    lnf_b = np.asarray(inputs["lnf_b"], f32)
    Whead = np.asarray(inputs["Whead"], f32)
    bhead = np.asarray(inputs["bhead"], f32)

    ntok = n_b * T
    scale = f32(D) ** -0.5

    idx_c = idx[core * n_b:(core + 1) * n_b].reshape(-1)         # [ntok]
    hot = (idx_c[None, :] == np.arange(V)[:, None]).astype(f32)  # [V, ntok]

    # center embeddings per token/position so x0 is feature-mean-free
    tok_emb = tok_emb - tok_emb.mean(axis=1, keepdims=True)
    pos_emb = pos_emb - pos_emb.mean(axis=1, keepdims=True)

    posT = pos_emb.T.astype(f32)                                 # [D, T]
    posT2 = np.concatenate([posT, posT], axis=1)                 # [D, 512]
    pos_in = posT2.reshape(KC, 128, 512).transpose(1, 0, 2).copy()

    lane = np.arange(128)
    t = np.arange(T)
    tri0 = (lane[:, None] <= t[None, :128]).astype(f32)          # diag block
    mask = np.concatenate([tri0, np.ones((128, 128), f32), tri0], axis=1)

    def pack_w(w):  # [D_in, N] -> [128, KC_in, N]
        kin = w.shape[0] // 128
        return w.reshape(kin, 128, -1).transpose(1, 0, 2).copy()

    wqkv = np.zeros((n_layers, 128, 3, KC, D), f32)
    wproj = np.zeros((n_layers, 128, KC, D), f32)
    w1 = np.zeros((n_layers, 128, KC, DFF), f32)
    w2 = np.zeros((n_layers, 128, FT, D), f32)
    vbias = np.zeros((n_layers, 128, D), f32)
    biasc = np.zeros((n_layers, 128, NBC), f32)

    # The feature-mean of the residual stream is kept identically zero by
    # column-centering every contribution to it (embeddings + proj/ffn2
    # outputs): LN is invariant to per-token mean shifts of its input, so
    # results are unchanged, and the kernel's LN needs no mean statistics.
    # Additionally every weight consuming a post-LN activation is centered
    # along its input dim: (x - mean(x)) @ Wg == x @ (Wg - colmean(Wg)).
    center_in = lambda w: w - w.mean(axis=0, keepdims=True)    # input dim
    center_out = lambda w: w - w.mean(axis=1, keepdims=True)   # output dim

    for l in range(n_layers):
        # Wq[l] is [H, D, HD]; feature f = h*HD+hd -> transpose to [D, H, HD]
        wq2 = Wq[l].transpose(1, 0, 2).reshape(D, D) * scale
        wk2 = Wk[l].transpose(1, 0, 2).reshape(D, D)
        wv2 = Wv[l].transpose(1, 0, 2).reshape(D, D)
        wqkv[l, :, 0] = pack_w(center_in(wq2 * ln1_g[l][:, None]))
        wqkv[l, :, 1] = pack_w(center_in(wk2 * ln1_g[l][:, None]))
        wqkv[l, :, 2] = pack_w(center_in(wv2 * ln1_g[l][:, None]))
        vbias[l] = np.broadcast_to(ln1_b[l] @ wv2, (128, D))
        wproj[l] = pack_w(center_out(Wproj[l]))
        w1[l] = pack_w(center_in(W1[l] * ln2_g[l][:, None]))
        w2[l] = pack_w(center_out(W2[l]))
        biasc[l, :, 0:MT] = (bproj[l] - bproj[l].mean()).reshape(MT, 128).T
        biasc[l, :, MT:MT + FT] = (b1[l] + ln2_b[l] @ W1[l]).reshape(FT, 128).T
        biasc[l, :, MT + FT:MT + FT + MT] = \
            (b2[l] - b2[l].mean()).reshape(MT, 128).T
        biasc[l, :, MT + FT + MT:MT + FT + 2 * MT] = \
            (ln1_b[l] @ wq2).reshape(MT, 128).T
        biasc[l, :, MT + FT + 2 * MT:] = (ln1_b[l] @ wk2).reshape(MT, 128).T

    whead_eff = center_in(Whead * lnf_g[:, None])
    bhead_eff = (bhead + lnf_b @ Whead).astype(f32)

    cst = np.ones((128, CST_W), f32)
    cst[:, 512] = 1.0 / D
    cst[:, 513:641] = np.eye(128, dtype=f32)
    cst[:, 641:769] = 1.0 / 512  # exact in bf16; 512/D folded into Ln scale

    bf = lambda a: np.ascontiguousarray(a).astype(NPBF)

    return {
        "cst": bf(cst),
        "epsc": np.full((128, 1), EPS, f32),
        "hotT": bf(hot),
        "temb": bf(tok_emb),
        "posT": bf(pos_in),
        "mask": bf(mask),
        "wqkv": bf(wqkv),
        "wproj": bf(wproj),
        "w1": bf(w1),
        "w2": bf(w2),
        "vbias": vbias,
        "biasc": biasc,
        "whead": bf(pack_w(whead_eff)),
        "bheadc": bhead_eff[:, None].copy(),
    }


_CACHE = {}


def get_program():
    if "nc" not in _CACHE:
        _CACHE["nc"] = build_program()
    return _CACHE["nc"]


def run_on_hw(inputs, trace=False):
    nc = get_program()
    in_maps = [prep_inputs(inputs, core=c) for c in range(NCORES)]
    res = run_bass_kernel_spmd(nc, in_maps, list(range(NCORES)), trace=trace)
    outs = []
    for c in range(NCORES):
        lt = res.results[c]["logitsT"]          # [BL, V, T]
        outs.append(lt.transpose(0, 2, 1))      # [BL, T, V]
    full = np.concatenate(outs, axis=0)         # [B, T, V]
    return full, res


def kernel(**inputs):
    out, _ = run_on_hw(inputs, trace=False)
    return out
